# revision 1
# baseline (speedup 1.0000x reference)
"""GCN layer on 8 NeuronCores — two-round batched dma_gather version.

Per layer, per core (dest-sharded, MC=102400 dests):
  Round 1: 32 InstDMAGatherAnt instructions (one per src%32 offset class,
    hbm_base = table + o*8, stride 256B, elem 8B) fetch each edge's source
    row into a class-blocked staging buffer (stream pos i -> SBUF
    [i%128, i//128]).  Host assigns each edge a staging position whose
    DRAM flat index (p1*S1 + s1, S1 = 1 mod 32) has residue == the edge's
    dest-sorted slot block, so that
  Round 2: after one contiguous SBUF->DRAM write, 32 more gathers (class k
    reads staged + k*8) land every message at its dest-sorted slot
    (p = dest partition, s = dest-sorted rank).  Zero page serves dummy
    (zero-degree) and pad slots.
  Aggregation: prefix scan + mask-cascade boundary extraction (unchanged).
  BatchNorm stats AllReduce'd; xs/ys tables AllGather'd.
"""

import numpy as np

N, T, V = 64, 512, 25
L = 2 * V
M = N * T * V            # 819200 nodes
P = 128
NCORES = 8
MC = M // NCORES         # 102400 dests per core
PD = MC // P             # 800 dests per partition
NPAGES = M // 32         # 25600 table pages
HID = 20
BN_EPS = 1e-5
PRE = 832
NQUEUES = 4

_runtime = {}


def _setup_runtime():
    if _runtime:
        return _runtime
    import concourse.bass as bass
    import concourse.tile as tile
    from concourse import mybir
    import bass_rust
    from concourse.vector_clock import ScopedClock, VectorClock

    def _split_drain_and_barrier(self, tick_clock, wait_clock):
        nc = self.nc
        gc = tick_clock.global_clock
        n = len(gc)
        for p in range(n):
            t = gc[p]
            if t > 0:
                vc = VectorClock([t if i == p else 0 for i in range(n)])
                carrier = nc.sync.nop()
                wait_clock.add_sem_waits(carrier.ins, ScopedClock({None: vc}))
        nc.sync.drain()
        nc.all_engine_barrier()
        assert self.sems is not None
        popped = nc._tile_sem_poison_stack.pop()
        assert popped is self._sem_poison
        nc.clear_and_free_semaphores(list(self.sems.allocated().values()))
        nc.all_engine_barrier()

    MAXW = 1

    def _split_waits_in_blocks(self, ordered_blocks):
        nc = self.nc
        for bb_name, insts in ordered_blocks.items():
            new_list = []
            for inst in insts:
                si = inst.sync_info
                waits = list(si.on_wait) if (si and si.on_wait) else []
                if len(waits) > MAXW:
                    keep = waits[:MAXW - 1]
                    excess = waits[MAXW - 1:]
                    for k in range(0, len(excess), MAXW):
                        chunk = excess[k:k + MAXW]
                        carrier = mybir.InstEventSemaphore(
                            name=f"WSPLIT-{nc.next_id()}", ins=[], outs=[])
                        carrier.engine = inst.engine
                        carrier.sync_info = mybir.SyncInfo(
                            on_wait=list(chunk), on_update=[])
                        carrier.debug = inst.debug
                        new_list.append(carrier)
                    inst.sync_info = mybir.SyncInfo(
                        on_wait=keep,
                        on_update=list(si.on_update) if si.on_update else [])
                new_list.append(inst)
            insts[:] = new_list

    _orig_lower = tile.TileContext._lower_ordered_insts

    def _patched_lower(self, postordered_blocks):
        _split_waits_in_blocks(self, postordered_blocks)
        return _orig_lower(self, postordered_blocks)

    tile.TileContext._drain_and_barrier = _split_drain_and_barrier
    if getattr(tile.TileContext._lower_ordered_insts, "__name__", "") != "_patched_lower":
        tile.TileContext._lower_ordered_insts = _patched_lower

    _runtime["bass"] = bass
    _runtime["tile"] = tile
    _runtime["mybir"] = mybir
    return _runtime


def _finalize_libraries(nc, mybir):
    import bass_rust
    from concourse.library_config import all_libraries, standard
    mask = {}
    for lib in all_libraries:
        for t in lib.instructions:
            mask[t] = mask.get(t, 0) | (1 << lib.index)
    bass_rust.insert_library_loads(nc, mask, len(all_libraries), standard.index)
    mybir.codegen_inst_isa_subclasses(nc)


# --------------------------------------------------------------------------
# host-side preprocessing (index manipulation only)
# --------------------------------------------------------------------------

def _cascade_masks(lptr, ES):
    """Baseline boundary-extraction cascade masks (see kernel.py)."""
    W = PRE
    WA = PRE + ES + 1
    nparts, npd1 = lptr.shape
    g = np.empty((nparts, W), np.int64)
    g[:, :npd1] = PRE + lptr
    g[:, npd1:] = (PRE + lptr[:, -1:]) + np.arange(1, W - npd1 + 1)[None, :]
    d = np.arange(W)[None, :]
    o = g - d
    assert (o >= 0).all() and int(g.max()) < WA
    nbits = max(1, int(np.ceil(np.log2(int(o.max()) + 1))))
    pos = np.broadcast_to(d, (nparts, W)).copy()
    rowoff = (np.arange(nparts) * WA)[:, None]
    masks_by_shift = {}
    for j in range(nbits - 1, -1, -1):
        b = ((o >> j) & 1).astype(np.uint8)
        lo = np.full(nparts * WA, 2, np.int8)
        hi = np.full(nparts * WA, -1, np.int8)
        flat = (rowoff + pos).ravel()
        np.minimum.at(lo, flat, b.ravel().astype(np.int8))
        np.maximum.at(hi, flat, b.ravel().astype(np.int8))
        used = hi >= 0
        assert (lo[used] == hi[used]).all(), "cascade routing conflict"
        m = np.zeros(nparts * WA, np.uint8)
        m[used] = hi[used].astype(np.uint8)
        masks_by_shift[1 << j] = m.reshape(nparts, WA)
        pos = pos + (b.astype(np.int64) << j)
    assert (pos == g).all()
    shifts = sorted(masks_by_shift)
    masks = [masks_by_shift[s] for s in shifts]
    return shifts, masks


def _wrap_stream(pages, width):
    """[n] int stream -> [128, width*8] int16 wrapped in 16, replicated x8."""
    n = len(pages)
    k16 = width * 8
    pad = np.zeros(k16 * 16, np.int16)
    pad[:n] = pages.astype(np.int16)
    w = pad.reshape(k16, 16).T          # [16, k16]
    return np.tile(w, (8, 1))           # [128, k16]


def _host_prep(edge_index):
    row = np.asarray(edge_index[0], dtype=np.int64)
    col = np.asarray(edge_index[1], dtype=np.int64)
    deg = np.bincount(col, minlength=M).astype(np.float32) + 1.0

    percore = []
    for k in range(NCORES):
        sel = (col >= k * MC) & (col < (k + 1) * MC)
        r = row[sel]
        c = col[sel] - k * MC
        dcnt = np.bincount(c, minlength=MC)
        zdest = np.nonzero(dcnt == 0)[0]
        r = np.concatenate([r, np.full(len(zdest), -1, np.int64)])
        c = np.concatenate([c, zdest])
        order = np.argsort(c, kind="stable")
        r, c = r[order], c[order]
        part = c // PD
        cnt = np.bincount(part, minlength=P)
        starts = np.concatenate([[0], np.cumsum(cnt)])
        lptr = np.zeros((P, PD + 1), np.int64)
        slot = np.empty(len(c), np.int64)
        for p in range(P):
            sl = slice(starts[p], starts[p + 1])
            loc = c[sl] - p * PD
            lptr[p] = np.searchsorted(loc, np.arange(PD + 1))
            slot[sl] = np.arange(starts[p + 1] - starts[p])
        percore.append(dict(r=r, part=part, slot=slot, cnt=cnt, lptr=lptr))

    ES = int(32 * np.ceil((max(pc["cnt"].max() for pc in percore) + 40) / 32))
    ES32 = ES // 32

    # uniform per-class round-1 lengths across cores
    dem_all = np.zeros((NCORES, 32, 32), np.int64)
    for k, pc in enumerate(percore):
        real = pc["r"] >= 0
        o = pc["r"][real] % 32
        kblk = pc["slot"][real] // ES32
        np.add.at(dem_all[k], (o, kblk), 1)
    Lo = np.ceil(dem_all.max(axis=(0, 2)) / 4).astype(np.int64)
    Lo = np.maximum(Lo, 1)
    Lo = 8 * np.ceil(Lo / 8).astype(np.int64)   # chunks of exactly 1024 idxs
    b = np.concatenate([[0], np.cumsum(Lo)])
    S1 = int(b[-1])
    S1 += (1 - S1) % 32                    # S1 = 1 (mod 32)
    ZP = 4 * S1                            # zero page index in staged buffer
    assert ZP + 1 < 32768 and NPAGES < 32768

    cores = []
    all_shifts = None
    for k, pc in enumerate(percore):
        r, part, slot = pc["r"], pc["part"], pc["slot"]
        real = r >= 0
        o = r[real] % 32
        src_page = r[real] >> 5
        kblk = slot[real] // ES32
        p_dest = part[real]
        s_dest = slot[real]
        # rank within (o, kblk) group
        order2 = np.lexsort((np.arange(o.size), kblk, o))
        oo, kk = o[order2], kblk[order2]
        grp = oo * 32 + kk
        first = np.concatenate([[True], grp[1:] != grp[:-1]])
        gidx = np.cumsum(first) - 1
        gstart = np.nonzero(first)[0]
        j = np.arange(o.size) - gstart[gidx]
        assert (j < 4 * Lo[oo]).all(), "round-1 class capacity exceeded"
        s1 = b[oo] + (j >> 2)
        p1 = ((kk - s1) % 32) + 32 * (j & 3)
        # round-1 idx stream: pos i1 = s1*128 + p1 -> table page
        pages1 = np.zeros(S1 * 128, np.int16)
        pages1[s1 * 128 + p1] = src_page[order2].astype(np.int16)
        # round-2: final (p, s) -> staged page
        flat1 = p1 * S1 + s1
        assert ((flat1 & 31) == kk).all()
        pages2 = np.full(ES * 128, ZP, np.int16)
        i2 = s_dest[order2] * 128 + p_dest[order2]
        pages2[i2] = (flat1 >> 5).astype(np.int16)
        shifts, masks = _cascade_masks(pc["lptr"], ES)
        degf = None  # filled below
        cores.append(dict(pages1=pages1, pages2=pages2, shifts=shifts,
                          masks=masks))
    all_shifts = sorted({s for cd in cores for s in cd["shifts"]})
    WA = PRE + ES + 1
    for k, cd in enumerate(cores):
        sh2m = dict(zip(cd["shifts"], cd["masks"]))
        zero = np.zeros((P, WA), np.uint8)
        cd["masks"] = [sh2m.get(s, zero) for s in all_shifts]
        cd["shifts"] = all_shifts
        cd["degf"] = deg[k * MC:(k + 1) * MC].reshape(P, PD)
        cd["widx1"] = _wrap_stream(cd.pop("pages1"), S1)
        cd["widx2"] = _wrap_stream(cd.pop("pages2"), ES)
    return ES, S1, tuple(Lo.tolist()), all_shifts, cores


# --------------------------------------------------------------------------
# device program
# --------------------------------------------------------------------------

_REG_CACHE = {}


def _num_idxs_reg(nc, n):
    cache = _REG_CACHE.setdefault(id(nc), {})
    if n not in cache:
        cache[n] = nc.gpsimd.to_reg(n)
    return cache[n]


def _emit_dma_gather(nc, mybir, out_ap, in_ap, idxs_ap, num_idxs, queue_num=0):
    """InstDMAGatherAnt with 8B elements (elem_size=2 f32, stride 256B)."""
    eng = nc.gpsimd
    _in_ap = eng.lower_ap_dma(in_ap, for_custom_bir_dma=True)
    _idxs_ap = eng.lower_ap(idxs_ap)
    _out_ap = eng.lower_ap(out_ap)
    return eng.add_instruction(
        mybir.InstDMAGatherAnt(
            name=nc.get_next_instruction_name(),
            ins=[*_in_ap, _idxs_ap,
                 eng.lower_val_access(_num_idxs_reg(nc, num_idxs))],
            outs=[_out_ap],
            transpose=False, num_idxs=num_idxs, elem_size=2,
            stride_bytes_256=1, gen_mode=0, single_packet=True,
            queue_num=queue_num, sbuf_tokens_per_rank=0,
            sbuf_free_dim_per_rank=0, sbuf_free_dim_pad_per_rank=0,
            sbuf_byte_offset=0,
        ))


def _build_program(ES, S1, Lo, shifts):
    rt = _setup_runtime()
    bass, tile, mybir = rt["bass"], rt["tile"], rt["mybir"]
    f32, i16, u8 = mybir.dt.float32, mybir.dt.int16, mybir.dt.uint8
    bf16 = mybir.dt.bfloat16
    AF = mybir.ActivationFunctionType
    ALU = mybir.AluOpType
    nc = bass.Bass(target_bir_lowering=False, num_swdge_queues=NQUEUES)

    WA = PRE + ES + 1
    ES32 = ES // 32
    bcls = np.concatenate([[0], np.cumsum(np.asarray(Lo))]).astype(int)

    xloc = nc.declare_dram_parameter("xloc", [P, PD, 2], f32, isOutput=False)
    degf = nc.declare_dram_parameter("degf", [P, PD], f32, isOutput=False)
    widx1 = nc.declare_dram_parameter("widx1", [P, S1 * 8], i16, isOutput=False)
    widx2 = nc.declare_dram_parameter("widx2", [P, ES * 8], i16, isOutput=False)
    bmasks = nc.declare_dram_parameter("bmasks", [len(shifts), P, WA], u8,
                                       isOutput=False)
    w1 = nc.declare_dram_parameter("w1", [2, HID], f32, isOutput=False)
    gamma = nc.declare_dram_parameter("gamma", [1, HID], f32, isOutput=False)
    beta = nc.declare_dram_parameter("beta", [1, HID], f32, isOutput=False)
    w2 = nc.declare_dram_parameter("w2", [HID, 2], f32, isOutput=False)
    b2 = nc.declare_dram_parameter("b2", [1, 2], f32, isOutput=False)
    out_ext = nc.declare_dram_parameter("out", [P, PD, 2], f32, isOutput=True)

    shard = nc.dram_tensor("shard", [MC * 2], f32)
    table = nc.dram_tensor("table", [M * 2], f32, addr_space="Shared")
    staged = nc.dram_tensor("staged", [S1 * 128 * 2 + 64], f32)
    bn_in = nc.dram_tensor("bn_in", [2 * HID], f32)
    bn_out = nc.dram_tensor("bn_out", [2 * HID], f32, addr_space="Shared")
    groups = [list(range(NCORES))]

    from concourse.masks import make_identity

    with tile.TileContext(nc) as tc:
        with (
            tc.tile_pool(name="big", bufs=1) as big,
            tc.tile_pool(name="gst", bufs=3) as gst,
            tc.tile_pool(name="small", bufs=1) as small,
            tc.tile_pool(name="ps", bufs=2, space="PSUM") as psp,
        ):
            widx1_t = big.tile([P, S1 * 8], i16)
            nc.sync.dma_start(out=widx1_t[:], in_=widx1[:])
            widx2_t = big.tile([P, ES * 8], i16)
            nc.sync.dma_start(out=widx2_t[:], in_=widx2[:])
            xl = big.tile([P, PD, 2], f32)
            nc.sync.dma_start(out=xl[:], in_=xloc[:])
            dg = big.tile([P, PD], f32)
            nc.sync.dma_start(out=dg[:], in_=degf[:])

            def part_bcast(ap):
                return bass.AP(tensor=ap.tensor, offset=ap.offset,
                               ap=[[0, P], *ap.ap])

            w1_t = small.tile([P, 2 * HID], f32)
            nc.sync.dma_start(out=w1_t[:], in_=part_bcast(w1[:, :]))
            w2_t = small.tile([P, HID * 2], f32)
            nc.sync.dma_start(out=w2_t[:], in_=part_bcast(w2[:, :]))
            gm_t = small.tile([P, HID], f32)
            nc.sync.dma_start(out=gm_t[:], in_=part_bcast(gamma[0, :]))
            bt_t = small.tile([P, HID], f32)
            nc.sync.dma_start(out=bt_t[:], in_=part_bcast(beta[0, :]))
            b2_t = small.tile([P, 2], f32)
            nc.sync.dma_start(out=b2_t[:], in_=part_bcast(b2[0, :]))

            # zero page of the staged buffer
            zpg = small.tile([1, 64], f32)
            nc.vector.memset(zpg[:], 0.0)
            nc.sync.dma_start(out=staged[S1 * 256:S1 * 256 + 64], in_=zpg[:])

            dinv = dg
            nc.scalar.activation(out=dinv[:], in_=dg[:], func=AF.Sqrt)
            nc.vector.reciprocal(out=dinv[:], in_=dinv[:])

            def bcast_pd2(t):
                a = t[:]
                return bass.AP(tensor=a.tensor, offset=a.offset,
                               ap=[a.ap[0], a.ap[1], [0, 2]])

            def mul_dinv(dst, src):
                nc.vector.tensor_tensor(out=dst[:], in0=src[:],
                                        in1=bcast_pd2(dinv), op=ALU.mult)

            stg1 = big.tile([P, S1, 2], f32)
            msg = big.tile([P, ES, 2], f32)
            A = big.tile([P, WA, 2], f32)
            agg = big.tile([P, PD, 2], f32)
            zero1 = small.tile([P, 2], f32)
            nc.vector.memset(zero1[:], 0.0)

            qctr = [0]

            def chunked_gather(dst, in_ap_fn, idxs_t, lo, hi):
                """gathers in <=8-slot (1024-idx) chunks, rotating queues."""
                s = lo
                while s < hi:
                    e = min(s + 8, hi)
                    _emit_dma_gather(
                        nc, mybir, dst[:, s:e, :], in_ap_fn(),
                        idxs_t[:, s * 8:e * 8], (e - s) * 128,
                        queue_num=qctr[0] % NQUEUES)
                    qctr[0] += 1
                    s = e

            def gather_layer():
                # round 1: table -> class-blocked staging
                for o in range(32):
                    in_ap = lambda o=o: bass.AP(
                        tensor=table[:].tensor, offset=o * 2,
                        ap=[[64, NPAGES], [1, 2]])
                    chunked_gather(stg1, in_ap, widx1_t,
                                   int(bcls[o]), int(bcls[o + 1]))
                # barrier: all round-1 gather DMAs landed in stg1
                nc.gpsimd.drain()
                # staging -> DRAM, on gpsimd so the drain orders it
                st_ap = bass.AP(tensor=staged[:].tensor, offset=0,
                                ap=[[S1 * 2, P], [1, S1 * 2]])
                nc.gpsimd.dma_start(out=st_ap, in_=stg1[:])
                nc.gpsimd.drain()
                # round 2: staged -> dest-sorted msg
                for kblk in range(32):
                    in_ap = lambda kblk=kblk: bass.AP(
                        tensor=staged[:].tensor, offset=kblk * 2,
                        ap=[[64, 4 * S1 + 1], [1, 2]])
                    chunked_gather(msg, in_ap, widx2_t,
                                   kblk * ES32, (kblk + 1) * ES32)
                # barrier: all round-2 DMAs landed; then touch a pad slot of
                # msg on gpsimd so tile orders the vector scan after this
                # point (cross-engine visibility of the gathered data).
                nc.gpsimd.drain()
                nc.gpsimd.memset(msg[:, ES - 1:ES, :], 0.0)

            def aggregate(own):
                nc.vector.memset(A[:, :PRE + 1, :], 0.0)
                for f in range(2):
                    ma = msg[:]
                    src = bass.AP(tensor=ma.tensor, offset=ma.offset + f,
                                  ap=[ma.ap[0], [2, ES]])
                    aa = A[:]
                    dst = bass.AP(tensor=aa.tensor,
                                  offset=aa.offset + (PRE + 1) * 2 + f,
                                  ap=[aa.ap[0], [2, ES]])
                    zb = bass.AP(tensor=zero1.tensor, offset=zero1[:].offset,
                                 ap=[zero1[:].ap[0], [0, ES]])
                    nc.vector.tensor_tensor_scan(
                        out=dst, data0=src, data1=zb, initial=0.0,
                        op0=ALU.add, op1=ALU.add)
                for si, s in enumerate(shifts):
                    wdt = WA - s
                    mt = gst.tile([P, WA], u8, tag="cmask")
                    nc.sync.dma_start(out=mt[:], in_=bmasks[si])
                    mm = mt[:, :wdt]
                    mba = bass.AP(tensor=mm.tensor, offset=mm.offset,
                                  ap=[mm.ap[0], mm.ap[1], [0, 2]])
                    nc.vector.copy_predicated(
                        out=A[:, 0:wdt, :], mask=mba, data=A[:, s:s + wdt, :])
                nc.vector.tensor_tensor(out=agg[:], in0=A[:, 1:PD + 1, :],
                                        in1=A[:, 0:PD, :], op=ALU.subtract)
                nc.vector.tensor_tensor(out=agg[:], in0=agg[:], in1=own[:],
                                        op=ALU.add)
                mul_dinv(agg, agg)

            def publish(src):
                nc.sync.dma_start(out=shard[:], in_=src[:])
                return nc.gpsimd.collective_compute(
                    "AllGather", ALU.bypass, replica_groups=groups,
                    ins=[shard[:]], outs=[table[:]])

            # =========== layer 1 ===========
            xs = xl
            mul_dinv(xs, xl)
            publish(xs)
            gather_layer()
            aggregate(xs)

            h = big.tile([P, HID, PD], bf16)
            ag = agg[:]
            a0 = bass.AP(tensor=ag.tensor, offset=ag.offset, ap=[ag.ap[0], [2, PD]])
            a1 = bass.AP(tensor=ag.tensor, offset=ag.offset + 1, ap=[ag.ap[0], [2, PD]])
            for j in range(HID):
                nc.scalar.activation(out=h[:, j, :], in_=a0, func=AF.Copy,
                                     scale=w1_t[:, j:j + 1])
                nc.vector.scalar_tensor_tensor(
                    out=h[:, j, :], in0=a1, scalar=w1_t[:, HID + j:HID + j + 1],
                    in1=h[:, j, :], op0=ALU.mult, op1=ALU.add)

            st = small.tile([P, 2 * HID], f32)
            nc.vector.tensor_reduce(out=st[:, :HID], in_=h[:],
                                    axis=mybir.AxisListType.X, op=ALU.add)
            sqscratch = small.tile([P, PD], f32)
            for j in range(HID):
                nc.scalar.activation(
                    out=sqscratch[:], in_=h[:, j, :], func=AF.Square,
                    accum_out=st[:, HID + j:HID + j + 1])
            ones = small.tile([P, 1], f32)
            nc.vector.memset(ones[:], 1.0)
            stp = psp.tile([P, 2 * HID], f32, space="PSUM")
            nc.tensor.matmul(out=stp[:1, :], lhsT=ones[:], rhs=st[:],
                             start=True, stop=True)
            sred = small.tile([1, 2 * HID], f32)
            nc.vector.tensor_copy(out=sred[:], in_=stp[:1, :])
            nc.sync.dma_start(out=bn_in[:], in_=sred[:])
            nc.gpsimd.collective_compute(
                "AllReduce", ALU.add, replica_groups=groups,
                ins=[bn_in[:]], outs=[bn_out[:]])
            sums = small.tile([P, 2 * HID], f32)
            nc.sync.dma_start(out=sums[:], in_=part_bcast(bn_out[:]))
            mv = small.tile([P, 2 * HID], f32)
            nc.vector.tensor_scalar_mul(mv[:, :HID], sums[:, :HID], 1.0 / M)
            nc.vector.tensor_scalar_mul(mv[:, HID:], sums[:, HID:], 1.0 / M)
            nc.vector.tensor_tensor(out=sums[:, :HID], in0=mv[:, :HID],
                                    in1=mv[:, :HID], op=ALU.mult)
            nc.vector.tensor_tensor(out=mv[:, HID:], in0=mv[:, HID:],
                                    in1=sums[:, :HID], op=ALU.subtract)
            sbn = small.tile([P, 2 * HID], f32)
            nc.vector.tensor_scalar_add(mv[:, HID:], mv[:, HID:], BN_EPS)
            nc.scalar.activation(out=sbn[:, :HID], in_=mv[:, HID:], func=AF.Sqrt)
            nc.vector.reciprocal(out=sbn[:, :HID], in_=sbn[:, :HID])
            nc.vector.tensor_tensor(out=sbn[:, :HID], in0=sbn[:, :HID],
                                    in1=gm_t[:], op=ALU.mult)
            nc.vector.tensor_tensor(out=sbn[:, HID:], in0=mv[:, :HID],
                                    in1=sbn[:, :HID], op=ALU.mult)
            nc.vector.tensor_tensor(out=sbn[:, HID:], in0=bt_t[:],
                                    in1=sbn[:, HID:], op=ALU.subtract)

            for j in range(HID):
                nc.scalar.activation(out=h[:, j, :], in_=h[:, j, :],
                                     func=AF.Relu,
                                     scale=sbn[:, j:j + 1],
                                     bias=sbn[:, HID + j:HID + j + 1])
            ys = big.tile([P, PD, 2], f32)
            yv = ys[:]
            for f in range(2):
                yf = bass.AP(tensor=yv.tensor, offset=yv.offset + f,
                             ap=[yv.ap[0], [2, PD]])
                nc.scalar.activation(out=yf, in_=h[:, 0, :], func=AF.Copy,
                                     scale=w2_t[:, f:f + 1])
                for j in range(1, HID):
                    nc.vector.scalar_tensor_tensor(
                        out=yf, in0=h[:, j, :],
                        scalar=w2_t[:, j * 2 + f:j * 2 + f + 1],
                        in1=yf, op0=ALU.mult, op1=ALU.add)
            mul_dinv(ys, ys)

            # =========== layer 2 ===========
            publish(ys)
            gather_layer()
            aggregate(ys)
            b2b = bass.AP(tensor=b2_t.tensor, offset=b2_t[:].offset,
                          ap=[b2_t[:].ap[0], [0, PD], [1, 2]])
            nc.vector.tensor_tensor(out=agg[:], in0=agg[:], in1=b2b, op=ALU.add)
            nc.sync.dma_start(out=out_ext[:], in_=agg[:])

    _finalize_libraries(nc, mybir)
    return nc


_prog_cache = {}
LAST_EXEC_NS = None


def _install_ntff_shim():
    import sys as _sys
    import types, contextlib, ctypes
    if "antenv.axon_hooks" in _sys.modules:
        return
    try:
        import antenv.axon_hooks  # noqa: F401
        return
    except ImportError:
        pass
    so_path = "/opt/axon/libaxon_pjrt.so"

    def _make_hook():
        lib = ctypes.CDLL(so_path)
        if not hasattr(lib, "axon_start_nrt_profile"):
            return None
        lib.axon_start_nrt_profile.argtypes = [
            ctypes.POINTER(ctypes.c_int64), ctypes.c_size_t]
        lib.axon_start_nrt_profile.restype = ctypes.c_int64
        lib.axon_stop_nrt_profile.argtypes = [ctypes.c_char_p]
        lib.axon_stop_nrt_profile.restype = ctypes.c_int64

        @contextlib.contextmanager
        def _hook_cm(output_dir, device_ids):
            import jax
            jax.devices()
            if device_ids:
                ids = (ctypes.c_int64 * len(device_ids))(*device_ids)
                rc = lib.axon_start_nrt_profile(ids, len(device_ids))
            else:
                rc = lib.axon_start_nrt_profile(None, 0)
            if rc != 0:
                raise RuntimeError(f"axon_start_nrt_profile rc={rc}")
            try:
                yield
            finally:
                lib.axon_stop_nrt_profile(str(output_dir).encode())

        return _hook_cm

    hook = [None]

    def get_axon_ntff_profile_hook():
        if hook[0] is None:
            hook[0] = _make_hook()
        return hook[0]

    mod = types.ModuleType("antenv.axon_hooks")
    mod.get_axon_ntff_profile_hook = get_axon_ntff_profile_hook
    mod.set_axon_ntff_profile_hook = lambda h: hook.__setitem__(0, h)
    _sys.modules["antenv.axon_hooks"] = mod


def kernel(x, edge_index, W1, b1, gamma, beta, W2, b2):
    global LAST_EXEC_NS
    import os
    from concourse.bass_utils import run_bass_kernel_spmd

    x = np.asarray(x)
    xf = x.reshape(M, 2).astype(np.float32)
    ES, S1, Lo, shifts, cores = _host_prep(np.asarray(edge_index))

    key = (ES, S1, Lo, tuple(shifts))
    if key not in _prog_cache:
        _prog_cache[key] = _build_program(ES, S1, Lo, shifts)
    nc = _prog_cache[key]

    in_maps = []
    for k in range(NCORES):
        cd = cores[k]
        in_maps.append({
            "xloc": xf[k * MC:(k + 1) * MC].reshape(P, PD, 2),
            "degf": cd["degf"],
            "widx1": cd["widx1"],
            "widx2": cd["widx2"],
            "bmasks": np.stack(cd["masks"]).astype(np.uint8),
            "w1": np.asarray(W1, np.float32),
            "gamma": np.asarray(gamma, np.float32).reshape(1, HID),
            "beta": np.asarray(beta, np.float32).reshape(1, HID),
            "w2": np.asarray(W2, np.float32),
            "b2": np.asarray(b2, np.float32).reshape(1, 2),
        })
    trace = os.environ.get("GCN_TRACE") == "1"
    if trace:
        _install_ntff_shim()
    res = None
    last_exc = None
    for attempt in range(3):
        try:
            res = run_bass_kernel_spmd(nc, in_maps, list(range(NCORES)),
                                       trace=trace)
            break
        except Exception as e:
            last_exc = e
            import time as _time
            _time.sleep(3.0)
    if res is None:
        raise last_exc
    if res.exec_time_ns is not None:
        LAST_EXEC_NS = res.exec_time_ns
    out = np.concatenate([res.results[k]["out"].reshape(MC, 2)
                          for k in range(NCORES)], axis=0)
    return out.reshape(N, T, L).astype(np.float32)



# revision 13
# speedup vs baseline: 1.7241x; 1.7241x over previous
"""GCN layer on 8 NeuronCores — two-round batched dma_gather version.

Per layer, per core (dest-sharded, MC=102400 dests):
  Round 1: 32 InstDMAGatherAnt instructions (one per src%32 offset class,
    hbm_base = table + o*8, stride 256B, elem 8B) fetch each edge's source
    row into a class-blocked staging buffer (stream pos i -> SBUF
    [i%128, i//128]).  Host assigns each edge a staging position whose
    DRAM flat index (p1*S1 + s1, S1 = 1 mod 32) has residue == the edge's
    dest-sorted slot block, so that
  Round 2: after one contiguous SBUF->DRAM write, 32 more gathers (class k
    reads staged + k*8) land every message at its dest-sorted slot
    (p = dest partition, s = dest-sorted rank).  Zero page serves dummy
    (zero-degree) and pad slots.
  Aggregation: prefix scan + mask-cascade boundary extraction (unchanged).
  BatchNorm stats AllReduce'd; xs/ys tables AllGather'd.
"""

import numpy as np

N, T, V = 64, 512, 25
L = 2 * V
M = N * T * V            # 819200 nodes
P = 128
NCORES = 8
MC = M // NCORES         # 102400 dests per core
PD = MC // P             # 800 dests per partition
NPAGES = M // 32         # 25600 table pages
HID = 20
BN_EPS = 1e-5
PRE = 832
NQUEUES = 4
CHUNK_SLOTS = 8            # slots (x128 idx) per gather instruction; HW caps num_idxs at 1024

_runtime = {}


def _setup_runtime():
    if _runtime:
        return _runtime
    import concourse.bass as bass
    import concourse.tile as tile
    from concourse import mybir
    import bass_rust
    from concourse.vector_clock import ScopedClock, VectorClock

    def _split_drain_and_barrier(self, tick_clock, wait_clock):
        nc = self.nc
        gc = tick_clock.global_clock
        n = len(gc)
        for p in range(n):
            t = gc[p]
            if t > 0:
                vc = VectorClock([t if i == p else 0 for i in range(n)])
                carrier = nc.sync.nop()
                wait_clock.add_sem_waits(carrier.ins, ScopedClock({None: vc}))
        nc.sync.drain()
        nc.all_engine_barrier()
        assert self.sems is not None
        popped = nc._tile_sem_poison_stack.pop()
        assert popped is self._sem_poison
        nc.clear_and_free_semaphores(list(self.sems.allocated().values()))
        nc.all_engine_barrier()

    MAXW = 1

    def _split_waits_in_blocks(self, ordered_blocks):
        nc = self.nc
        for bb_name, insts in ordered_blocks.items():
            new_list = []
            for inst in insts:
                si = inst.sync_info
                waits = list(si.on_wait) if (si and si.on_wait) else []
                if len(waits) > MAXW:
                    keep = waits[:MAXW - 1]
                    excess = waits[MAXW - 1:]
                    for k in range(0, len(excess), MAXW):
                        chunk = excess[k:k + MAXW]
                        carrier = mybir.InstEventSemaphore(
                            name=f"WSPLIT-{nc.next_id()}", ins=[], outs=[])
                        carrier.engine = inst.engine
                        carrier.sync_info = mybir.SyncInfo(
                            on_wait=list(chunk), on_update=[])
                        carrier.debug = inst.debug
                        new_list.append(carrier)
                    inst.sync_info = mybir.SyncInfo(
                        on_wait=keep,
                        on_update=list(si.on_update) if si.on_update else [])
                new_list.append(inst)
            insts[:] = new_list

    _orig_lower = tile.TileContext._lower_ordered_insts

    def _patched_lower(self, postordered_blocks):
        _split_waits_in_blocks(self, postordered_blocks)
        return _orig_lower(self, postordered_blocks)

    tile.TileContext._drain_and_barrier = _split_drain_and_barrier
    if getattr(tile.TileContext._lower_ordered_insts, "__name__", "") != "_patched_lower":
        tile.TileContext._lower_ordered_insts = _patched_lower

    _runtime["bass"] = bass
    _runtime["tile"] = tile
    _runtime["mybir"] = mybir
    return _runtime


def _finalize_libraries(nc, mybir):
    import bass_rust
    from concourse.library_config import all_libraries, standard
    mask = {}
    for lib in all_libraries:
        for t in lib.instructions:
            mask[t] = mask.get(t, 0) | (1 << lib.index)
    bass_rust.insert_library_loads(nc, mask, len(all_libraries), standard.index)
    mybir.codegen_inst_isa_subclasses(nc)


# --------------------------------------------------------------------------
# host-side preprocessing (index manipulation only)
# --------------------------------------------------------------------------

def _cascade_masks(lptr, ES):
    """Baseline boundary-extraction cascade masks (see kernel.py)."""
    W = PRE
    WA = PRE + ES + 1
    nparts, npd1 = lptr.shape
    g = np.empty((nparts, W), np.int64)
    g[:, :npd1] = PRE + lptr
    g[:, npd1:] = (PRE + lptr[:, -1:]) + np.arange(1, W - npd1 + 1)[None, :]
    d = np.arange(W)[None, :]
    o = g - d
    assert (o >= 0).all() and int(g.max()) < WA
    nbits = max(1, int(np.ceil(np.log2(int(o.max()) + 1))))
    pos = np.broadcast_to(d, (nparts, W)).copy()
    rowoff = (np.arange(nparts) * WA)[:, None]
    masks_by_shift = {}
    for j in range(nbits - 1, -1, -1):
        b = ((o >> j) & 1).astype(np.uint8)
        lo = np.full(nparts * WA, 2, np.int8)
        hi = np.full(nparts * WA, -1, np.int8)
        flat = (rowoff + pos).ravel()
        np.minimum.at(lo, flat, b.ravel().astype(np.int8))
        np.maximum.at(hi, flat, b.ravel().astype(np.int8))
        used = hi >= 0
        assert (lo[used] == hi[used]).all(), "cascade routing conflict"
        m = np.zeros(nparts * WA, np.uint8)
        m[used] = hi[used].astype(np.uint8)
        masks_by_shift[1 << j] = m.reshape(nparts, WA)
        pos = pos + (b.astype(np.int64) << j)
    assert (pos == g).all()
    shifts = sorted(masks_by_shift)
    masks = [masks_by_shift[s] for s in shifts]
    return shifts, masks


def _wrap_stream(pages, width):
    """[n] int stream -> [128, width*8] int16 wrapped in 16, replicated x8."""
    n = len(pages)
    k16 = width * 8
    pad = np.zeros(k16 * 16, np.int16)
    pad[:n] = pages.astype(np.int16)
    w = pad.reshape(k16, 16).T          # [16, k16]
    return np.tile(w, (8, 1))           # [128, k16]


def _host_prep(edge_index):
    row = np.asarray(edge_index[0], dtype=np.int64)
    col = np.asarray(edge_index[1], dtype=np.int64)
    deg = np.bincount(col, minlength=M).astype(np.float32) + 1.0

    percore = []
    for k in range(NCORES):
        sel = (col >= k * MC) & (col < (k + 1) * MC)
        r = row[sel]
        c = col[sel] - k * MC
        dcnt = np.bincount(c, minlength=MC)
        zdest = np.nonzero(dcnt == 0)[0]
        r = np.concatenate([r, np.full(len(zdest), -1, np.int64)])
        c = np.concatenate([c, zdest])
        order = np.argsort(c, kind="stable")
        r, c = r[order], c[order]
        part = c // PD
        cnt = np.bincount(part, minlength=P)
        starts = np.concatenate([[0], np.cumsum(cnt)])
        lptr = np.zeros((P, PD + 1), np.int64)
        slot = np.empty(len(c), np.int64)
        for p in range(P):
            sl = slice(starts[p], starts[p + 1])
            loc = c[sl] - p * PD
            lptr[p] = np.searchsorted(loc, np.arange(PD + 1))
            slot[sl] = np.arange(starts[p + 1] - starts[p])
        percore.append(dict(r=r, part=part, slot=slot, cnt=cnt, lptr=lptr))

    ES = int(32 * np.ceil((max(pc["cnt"].max() for pc in percore) + 40) / 32))
    ES32 = ES // 32

    # uniform per-class round-1 lengths across cores
    dem_all = np.zeros((NCORES, 32, 32), np.int64)
    for k, pc in enumerate(percore):
        real = pc["r"] >= 0
        o = pc["r"][real] % 32
        kblk = pc["slot"][real] // ES32
        np.add.at(dem_all[k], (o, kblk), 1)
    Lo = np.ceil(dem_all.max(axis=(0, 2)) / 4).astype(np.int64)
    Lo = np.maximum(Lo, 1)
    b = np.concatenate([[0], np.cumsum(Lo)])
    S1 = int(b[-1])
    S1 += (1 - S1) % 32                    # S1 = 1 (mod 32)
    ZP = 4 * S1                            # first of 32 zero pages in staged
    assert ZP + 32 < 32768 and NPAGES < 32768

    cores = []
    all_shifts = None
    for k, pc in enumerate(percore):
        r, part, slot = pc["r"], pc["part"], pc["slot"]
        real = r >= 0
        o = r[real] % 32
        src_page = r[real] >> 5
        kblk = slot[real] // ES32
        p_dest = part[real]
        s_dest = slot[real]
        # rank within (o, kblk) group
        order2 = np.lexsort((np.arange(o.size), kblk, o))
        oo, kk = o[order2], kblk[order2]
        grp = oo * 32 + kk
        first = np.concatenate([[True], grp[1:] != grp[:-1]])
        gidx = np.cumsum(first) - 1
        gstart = np.nonzero(first)[0]
        j = np.arange(o.size) - gstart[gidx]
        assert (j < 4 * Lo[oo]).all(), "round-1 class capacity exceeded"
        s1 = b[oo] + (j >> 2)
        p1 = ((kk - s1) % 32) + 32 * (j & 3)
        # round-1 idx stream: pos i1 = s1*128 + p1 -> table page.
        # pad positions gather unused data; spread them over random pages
        # (a single shared pad page serializes on one DRAM bank).
        prng = np.random.default_rng(12345 + k)
        pages1 = prng.integers(0, NPAGES, S1 * 128).astype(np.int16)
        pages1[s1 * 128 + p1] = src_page[order2].astype(np.int16)
        # round-2: final (p, s) -> staged page; pads cycle over 32 zero pages
        flat1 = p1 * S1 + s1
        assert ((flat1 & 31) == kk).all()
        pages2 = (ZP + (np.arange(ES * 128) & 31)).astype(np.int16)
        i2 = s_dest[order2] * 128 + p_dest[order2]
        pages2[i2] = (flat1 >> 5).astype(np.int16)
        shifts, masks = _cascade_masks(pc["lptr"], ES)
        degf = None  # filled below
        cores.append(dict(pages1=pages1, pages2=pages2, shifts=shifts,
                          masks=masks))
    all_shifts = sorted({s for cd in cores for s in cd["shifts"]})
    WA = PRE + ES + 1
    for k, cd in enumerate(cores):
        sh2m = dict(zip(cd["shifts"], cd["masks"]))
        zero = np.zeros((P, WA), np.uint8)
        cd["masks"] = [sh2m.get(s, zero) for s in all_shifts]
        cd["shifts"] = all_shifts
        cd["degf"] = deg[k * MC:(k + 1) * MC].reshape(P, PD)
        cd["widx1"] = _wrap_stream(cd.pop("pages1"), S1)
        cd["widx2"] = _wrap_stream(cd.pop("pages2"), ES)
    return ES, S1, tuple(Lo.tolist()), all_shifts, cores


# --------------------------------------------------------------------------
# device program
# --------------------------------------------------------------------------

_REG_CACHE = {}


def _num_idxs_reg(nc, n):
    cache = _REG_CACHE.setdefault(id(nc), {})
    if n not in cache:
        cache[n] = nc.gpsimd.to_reg(n)
    return cache[n]


def _emit_dma_gather(nc, mybir, out_ap, in_ap, idxs_ap, num_idxs, queue_num=0):
    """InstDMAGatherAnt with 8B elements (elem_size=2 f32, stride 256B)."""
    eng = nc.gpsimd
    _in_ap = eng.lower_ap_dma(in_ap, for_custom_bir_dma=True)
    _idxs_ap = eng.lower_ap(idxs_ap)
    _out_ap = eng.lower_ap(out_ap)
    return eng.add_instruction(
        mybir.InstDMAGatherAnt(
            name=nc.get_next_instruction_name(),
            ins=[*_in_ap, _idxs_ap,
                 eng.lower_val_access(_num_idxs_reg(nc, num_idxs))],
            outs=[_out_ap],
            transpose=False, num_idxs=num_idxs, elem_size=2,
            stride_bytes_256=1, gen_mode=0, single_packet=True,
            queue_num=queue_num, sbuf_tokens_per_rank=0,
            sbuf_free_dim_per_rank=0, sbuf_free_dim_pad_per_rank=0,
            sbuf_byte_offset=0,
        ))


def _build_program(ES, S1, Lo, shifts):
    rt = _setup_runtime()
    bass, tile, mybir = rt["bass"], rt["tile"], rt["mybir"]
    f32, i16, u8 = mybir.dt.float32, mybir.dt.int16, mybir.dt.uint8
    bf16 = mybir.dt.bfloat16
    AF = mybir.ActivationFunctionType
    ALU = mybir.AluOpType
    nc = bass.Bass(target_bir_lowering=False, num_swdge_queues=NQUEUES)

    WA = PRE + ES + 1
    ES32 = ES // 32
    bcls = np.concatenate([[0], np.cumsum(np.asarray(Lo))]).astype(int)

    xloc = nc.declare_dram_parameter("xloc", [P, PD, 2], f32, isOutput=False)
    degf = nc.declare_dram_parameter("degf", [P, PD], f32, isOutput=False)
    widx1 = nc.declare_dram_parameter("widx1", [P, S1 * 8], i16, isOutput=False)
    widx2 = nc.declare_dram_parameter("widx2", [P, ES * 8], i16, isOutput=False)
    bmasks = nc.declare_dram_parameter("bmasks", [len(shifts), P, WA], u8,
                                       isOutput=False)
    w1 = nc.declare_dram_parameter("w1", [2, HID], f32, isOutput=False)
    gamma = nc.declare_dram_parameter("gamma", [1, HID], f32, isOutput=False)
    beta = nc.declare_dram_parameter("beta", [1, HID], f32, isOutput=False)
    w2 = nc.declare_dram_parameter("w2", [HID, 2], f32, isOutput=False)
    b2 = nc.declare_dram_parameter("b2", [1, 2], f32, isOutput=False)
    out_ext = nc.declare_dram_parameter("out", [P, PD, 2], f32, isOutput=True)

    shard = nc.dram_tensor("shard", [MC * 2], f32)
    table = nc.dram_tensor("table", [M * 2], f32, addr_space="Shared")
    staged = nc.dram_tensor("staged", [S1 * 128 * 2 + 64 * 32], f32)
    bn_in = nc.dram_tensor("bn_in", [2 * HID], f32)
    bn_out = nc.dram_tensor("bn_out", [2 * HID], f32, addr_space="Shared")
    groups = [list(range(NCORES))]

    from concourse.masks import make_identity

    with tile.TileContext(nc) as tc:
        with (
            tc.tile_pool(name="big", bufs=1) as big,
            tc.tile_pool(name="gst", bufs=3) as gst,
            tc.tile_pool(name="small", bufs=1) as small,
            tc.tile_pool(name="ps", bufs=2, space="PSUM") as psp,
        ):
            widx1_t = big.tile([P, S1 * 8], i16)
            nc.sync.dma_start(out=widx1_t[:], in_=widx1[:])
            widx2_t = big.tile([P, ES * 8], i16)
            nc.sync.dma_start(out=widx2_t[:], in_=widx2[:])
            xl = big.tile([P, PD, 2], f32)
            nc.sync.dma_start(out=xl[:], in_=xloc[:])
            dg = big.tile([P, PD], f32)
            nc.sync.dma_start(out=dg[:], in_=degf[:])

            def part_bcast(ap):
                return bass.AP(tensor=ap.tensor, offset=ap.offset,
                               ap=[[0, P], *ap.ap])

            w1_t = small.tile([P, 2 * HID], f32)
            nc.sync.dma_start(out=w1_t[:], in_=part_bcast(w1[:, :]))
            w2_t = small.tile([P, HID * 2], f32)
            nc.sync.dma_start(out=w2_t[:], in_=part_bcast(w2[:, :]))
            gm_t = small.tile([P, HID], f32)
            nc.sync.dma_start(out=gm_t[:], in_=part_bcast(gamma[0, :]))
            bt_t = small.tile([P, HID], f32)
            nc.sync.dma_start(out=bt_t[:], in_=part_bcast(beta[0, :]))
            b2_t = small.tile([P, 2], f32)
            nc.sync.dma_start(out=b2_t[:], in_=part_bcast(b2[0, :]))

            # 32 zero pages of the staged buffer
            zpg = small.tile([128, 16], f32)
            nc.vector.memset(zpg[:], 0.0)
            nc.sync.dma_start(out=staged[S1 * 256:S1 * 256 + 64 * 32],
                              in_=zpg[:])

            dinv = dg
            nc.scalar.activation(out=dinv[:], in_=dg[:], func=AF.Sqrt)
            nc.vector.reciprocal(out=dinv[:], in_=dinv[:])

            def bcast_pd2(t):
                a = t[:]
                return bass.AP(tensor=a.tensor, offset=a.offset,
                               ap=[a.ap[0], a.ap[1], [0, 2]])

            def mul_dinv(dst, src):
                nc.vector.tensor_tensor(out=dst[:], in0=src[:],
                                        in1=bcast_pd2(dinv), op=ALU.mult)

            stg1 = big.tile([P, S1, 2], f32)
            msg = big.tile([P, ES, 2], f32)
            A = big.tile([P, WA, 2], f32)
            agg = big.tile([P, PD, 2], f32)
            zero1 = small.tile([P, 2], f32)
            nc.vector.memset(zero1[:], 0.0)

            qctr = [0]

            def chunked_gather(dst, in_ap_fn, idxs_t, lo, hi):
                """gathers in <=CHUNK_SLOTS chunks, rotating queues."""
                s = lo
                while s < hi:
                    e = min(s + CHUNK_SLOTS, hi)
                    _emit_dma_gather(
                        nc, mybir, dst[:, s:e, :], in_ap_fn(),
                        idxs_t[:, s * 8:e * 8], (e - s) * 128,
                        queue_num=qctr[0] % NQUEUES)
                    qctr[0] += 1
                    s = e

            def gather_layer():
                # round 1: table -> class-blocked staging
                for o in range(32):
                    in_ap = lambda o=o: bass.AP(
                        tensor=table[:].tensor, offset=o * 2,
                        ap=[[64, NPAGES], [1, 2]])
                    chunked_gather(stg1, in_ap, widx1_t,
                                   int(bcls[o]), int(bcls[o + 1]))
                # barrier: all round-1 gather DMAs landed in stg1
                nc.gpsimd.drain()
                # staging -> DRAM, on gpsimd so the drain orders it
                st_ap = bass.AP(tensor=staged[:].tensor, offset=0,
                                ap=[[S1 * 2, P], [1, S1 * 2]])
                nc.gpsimd.dma_start(out=st_ap, in_=stg1[:])
                nc.gpsimd.drain()
                # round 2: staged -> dest-sorted msg
                for kblk in range(32):
                    in_ap = lambda kblk=kblk: bass.AP(
                        tensor=staged[:].tensor, offset=kblk * 2,
                        ap=[[64, 4 * S1 + 32], [1, 2]])
                    chunked_gather(msg, in_ap, widx2_t,
                                   kblk * ES32, (kblk + 1) * ES32)
                # barrier: all round-2 DMAs landed; then touch a pad slot of
                # msg on gpsimd so tile orders the vector scan after this
                # point (cross-engine visibility of the gathered data).
                nc.gpsimd.drain()
                nc.gpsimd.memset(msg[:, ES - 1:ES, :], 0.0)

            def aggregate(own):
                nc.vector.memset(A[:, :PRE + 1, :], 0.0)
                for f in range(2):
                    ma = msg[:]
                    src = bass.AP(tensor=ma.tensor, offset=ma.offset + f,
                                  ap=[ma.ap[0], [2, ES]])
                    aa = A[:]
                    dst = bass.AP(tensor=aa.tensor,
                                  offset=aa.offset + (PRE + 1) * 2 + f,
                                  ap=[aa.ap[0], [2, ES]])
                    zb = bass.AP(tensor=zero1.tensor, offset=zero1[:].offset,
                                 ap=[zero1[:].ap[0], [0, ES]])
                    nc.vector.tensor_tensor_scan(
                        out=dst, data0=src, data1=zb, initial=0.0,
                        op0=ALU.add, op1=ALU.add)
                for si, s in enumerate(shifts):
                    wdt = WA - s
                    mt = gst.tile([P, WA], u8, tag="cmask")
                    nc.sync.dma_start(out=mt[:], in_=bmasks[si])
                    mm = mt[:, :wdt]
                    mba = bass.AP(tensor=mm.tensor, offset=mm.offset,
                                  ap=[mm.ap[0], mm.ap[1], [0, 2]])
                    nc.vector.copy_predicated(
                        out=A[:, 0:wdt, :], mask=mba, data=A[:, s:s + wdt, :])
                nc.vector.tensor_tensor(out=agg[:], in0=A[:, 1:PD + 1, :],
                                        in1=A[:, 0:PD, :], op=ALU.subtract)
                nc.vector.tensor_tensor(out=agg[:], in0=agg[:], in1=own[:],
                                        op=ALU.add)
                mul_dinv(agg, agg)

            def publish(src):
                nc.sync.dma_start(out=shard[:], in_=src[:])
                return nc.gpsimd.collective_compute(
                    "AllGather", ALU.bypass, replica_groups=groups,
                    ins=[shard[:]], outs=[table[:]])

            # =========== layer 1 ===========
            xs = xl
            mul_dinv(xs, xl)
            publish(xs)
            gather_layer()
            aggregate(xs)

            h = big.tile([P, HID, PD], bf16)
            ag = agg[:]
            a0 = bass.AP(tensor=ag.tensor, offset=ag.offset, ap=[ag.ap[0], [2, PD]])
            a1 = bass.AP(tensor=ag.tensor, offset=ag.offset + 1, ap=[ag.ap[0], [2, PD]])
            for j in range(HID):
                nc.scalar.activation(out=h[:, j, :], in_=a0, func=AF.Copy,
                                     scale=w1_t[:, j:j + 1])
                nc.vector.scalar_tensor_tensor(
                    out=h[:, j, :], in0=a1, scalar=w1_t[:, HID + j:HID + j + 1],
                    in1=h[:, j, :], op0=ALU.mult, op1=ALU.add)

            st = small.tile([P, 2 * HID], f32)
            nc.vector.tensor_reduce(out=st[:, :HID], in_=h[:],
                                    axis=mybir.AxisListType.X, op=ALU.add)
            sqscratch = small.tile([P, PD], f32)
            for j in range(HID):
                nc.scalar.activation(
                    out=sqscratch[:], in_=h[:, j, :], func=AF.Square,
                    accum_out=st[:, HID + j:HID + j + 1])
            ones = small.tile([P, 1], f32)
            nc.vector.memset(ones[:], 1.0)
            stp = psp.tile([P, 2 * HID], f32, space="PSUM")
            nc.tensor.matmul(out=stp[:1, :], lhsT=ones[:], rhs=st[:],
                             start=True, stop=True)
            sred = small.tile([1, 2 * HID], f32)
            nc.vector.tensor_copy(out=sred[:], in_=stp[:1, :])
            nc.sync.dma_start(out=bn_in[:], in_=sred[:])
            nc.gpsimd.collective_compute(
                "AllReduce", ALU.add, replica_groups=groups,
                ins=[bn_in[:]], outs=[bn_out[:]])
            sums = small.tile([P, 2 * HID], f32)
            nc.sync.dma_start(out=sums[:], in_=part_bcast(bn_out[:]))
            mv = small.tile([P, 2 * HID], f32)
            nc.vector.tensor_scalar_mul(mv[:, :HID], sums[:, :HID], 1.0 / M)
            nc.vector.tensor_scalar_mul(mv[:, HID:], sums[:, HID:], 1.0 / M)
            nc.vector.tensor_tensor(out=sums[:, :HID], in0=mv[:, :HID],
                                    in1=mv[:, :HID], op=ALU.mult)
            nc.vector.tensor_tensor(out=mv[:, HID:], in0=mv[:, HID:],
                                    in1=sums[:, :HID], op=ALU.subtract)
            sbn = small.tile([P, 2 * HID], f32)
            nc.vector.tensor_scalar_add(mv[:, HID:], mv[:, HID:], BN_EPS)
            nc.scalar.activation(out=sbn[:, :HID], in_=mv[:, HID:], func=AF.Sqrt)
            nc.vector.reciprocal(out=sbn[:, :HID], in_=sbn[:, :HID])
            nc.vector.tensor_tensor(out=sbn[:, :HID], in0=sbn[:, :HID],
                                    in1=gm_t[:], op=ALU.mult)
            nc.vector.tensor_tensor(out=sbn[:, HID:], in0=mv[:, :HID],
                                    in1=sbn[:, :HID], op=ALU.mult)
            nc.vector.tensor_tensor(out=sbn[:, HID:], in0=bt_t[:],
                                    in1=sbn[:, HID:], op=ALU.subtract)

            for j in range(HID):
                nc.scalar.activation(out=h[:, j, :], in_=h[:, j, :],
                                     func=AF.Relu,
                                     scale=sbn[:, j:j + 1],
                                     bias=sbn[:, HID + j:HID + j + 1])
            ys = big.tile([P, PD, 2], f32)
            yv = ys[:]
            for f in range(2):
                yf = bass.AP(tensor=yv.tensor, offset=yv.offset + f,
                             ap=[yv.ap[0], [2, PD]])
                nc.scalar.activation(out=yf, in_=h[:, 0, :], func=AF.Copy,
                                     scale=w2_t[:, f:f + 1])
                for j in range(1, HID):
                    nc.vector.scalar_tensor_tensor(
                        out=yf, in0=h[:, j, :],
                        scalar=w2_t[:, j * 2 + f:j * 2 + f + 1],
                        in1=yf, op0=ALU.mult, op1=ALU.add)
            mul_dinv(ys, ys)

            # =========== layer 2 ===========
            publish(ys)
            gather_layer()
            aggregate(ys)
            b2b = bass.AP(tensor=b2_t.tensor, offset=b2_t[:].offset,
                          ap=[b2_t[:].ap[0], [0, PD], [1, 2]])
            nc.vector.tensor_tensor(out=agg[:], in0=agg[:], in1=b2b, op=ALU.add)
            nc.sync.dma_start(out=out_ext[:], in_=agg[:])

    _finalize_libraries(nc, mybir)
    return nc


_prog_cache = {}
LAST_EXEC_NS = None


def _install_ntff_shim():
    import sys as _sys
    import types, contextlib, ctypes
    if "antenv.axon_hooks" in _sys.modules:
        return
    try:
        import antenv.axon_hooks  # noqa: F401
        return
    except ImportError:
        pass
    so_path = "/opt/axon/libaxon_pjrt.so"

    def _make_hook():
        lib = ctypes.CDLL(so_path)
        if not hasattr(lib, "axon_start_nrt_profile"):
            return None
        lib.axon_start_nrt_profile.argtypes = [
            ctypes.POINTER(ctypes.c_int64), ctypes.c_size_t]
        lib.axon_start_nrt_profile.restype = ctypes.c_int64
        lib.axon_stop_nrt_profile.argtypes = [ctypes.c_char_p]
        lib.axon_stop_nrt_profile.restype = ctypes.c_int64

        @contextlib.contextmanager
        def _hook_cm(output_dir, device_ids):
            import jax
            jax.devices()
            if device_ids:
                ids = (ctypes.c_int64 * len(device_ids))(*device_ids)
                rc = lib.axon_start_nrt_profile(ids, len(device_ids))
            else:
                rc = lib.axon_start_nrt_profile(None, 0)
            if rc != 0:
                raise RuntimeError(f"axon_start_nrt_profile rc={rc}")
            try:
                yield
            finally:
                lib.axon_stop_nrt_profile(str(output_dir).encode())

        return _hook_cm

    hook = [None]

    def get_axon_ntff_profile_hook():
        if hook[0] is None:
            hook[0] = _make_hook()
        return hook[0]

    mod = types.ModuleType("antenv.axon_hooks")
    mod.get_axon_ntff_profile_hook = get_axon_ntff_profile_hook
    mod.set_axon_ntff_profile_hook = lambda h: hook.__setitem__(0, h)
    _sys.modules["antenv.axon_hooks"] = mod


def kernel(x, edge_index, W1, b1, gamma, beta, W2, b2):
    global LAST_EXEC_NS
    import os
    from concourse.bass_utils import run_bass_kernel_spmd

    x = np.asarray(x)
    xf = x.reshape(M, 2).astype(np.float32)
    ES, S1, Lo, shifts, cores = _host_prep(np.asarray(edge_index))

    key = (ES, S1, Lo, tuple(shifts))
    if key not in _prog_cache:
        _prog_cache[key] = _build_program(ES, S1, Lo, shifts)
    nc = _prog_cache[key]

    in_maps = []
    for k in range(NCORES):
        cd = cores[k]
        in_maps.append({
            "xloc": xf[k * MC:(k + 1) * MC].reshape(P, PD, 2),
            "degf": cd["degf"],
            "widx1": cd["widx1"],
            "widx2": cd["widx2"],
            "bmasks": np.stack(cd["masks"]).astype(np.uint8),
            "w1": np.asarray(W1, np.float32),
            "gamma": np.asarray(gamma, np.float32).reshape(1, HID),
            "beta": np.asarray(beta, np.float32).reshape(1, HID),
            "w2": np.asarray(W2, np.float32),
            "b2": np.asarray(b2, np.float32).reshape(1, 2),
        })
    trace = os.environ.get("GCN_TRACE") == "1"
    if trace:
        _install_ntff_shim()
    res = None
    last_exc = None
    for attempt in range(3):
        try:
            res = run_bass_kernel_spmd(nc, in_maps, list(range(NCORES)),
                                       trace=trace)
            break
        except Exception as e:
            last_exc = e
            import time as _time
            _time.sleep(3.0)
    if res is None:
        raise last_exc
    if res.exec_time_ns is not None:
        LAST_EXEC_NS = res.exec_time_ns
    out = np.concatenate([res.results[k]["out"].reshape(MC, 2)
                          for k in range(NCORES)], axis=0)
    return out.reshape(N, T, L).astype(np.float32)



# revision 22
# speedup vs baseline: 1.7476x; 1.0136x over previous
"""GCN layer on 8 NeuronCores — two-round batched dma_gather version.

Per layer, per core (dest-sharded, MC=102400 dests):
  Round 1: 32 InstDMAGatherAnt instructions (one per src%32 offset class,
    hbm_base = table + o*8, stride 256B, elem 8B) fetch each edge's source
    row into a class-blocked staging buffer (stream pos i -> SBUF
    [i%128, i//128]).  Host assigns each edge a staging position whose
    DRAM flat index (p1*S1 + s1, S1 = 1 mod 32) has residue == the edge's
    dest-sorted slot block, so that
  Round 2: after one contiguous SBUF->DRAM write, 32 more gathers (class k
    reads staged + k*8) land every message at its dest-sorted slot
    (p = dest partition, s = dest-sorted rank).  Zero page serves dummy
    (zero-degree) and pad slots.
  Aggregation: prefix scan + mask-cascade boundary extraction (unchanged).
  BatchNorm stats AllReduce'd; xs/ys tables AllGather'd.
"""

import numpy as np

N, T, V = 64, 512, 25
L = 2 * V
M = N * T * V            # 819200 nodes
P = 128
NCORES = 8
MC = M // NCORES         # 102400 dests per core
PD = MC // P             # 800 dests per partition
NPAGES = M // 32         # 25600 table pages
HID = 20
BN_EPS = 1e-5
PRE = 832
NQUEUES = 4
CHUNK_SLOTS = 8            # slots (x128 idx) per gather instruction; HW caps num_idxs at 1024

_runtime = {}


def _setup_runtime():
    if _runtime:
        return _runtime
    import concourse.bass as bass
    import concourse.tile as tile
    from concourse import mybir
    import bass_rust
    from concourse.vector_clock import ScopedClock, VectorClock

    def _split_drain_and_barrier(self, tick_clock, wait_clock):
        nc = self.nc
        gc = tick_clock.global_clock
        n = len(gc)
        for p in range(n):
            t = gc[p]
            if t > 0:
                vc = VectorClock([t if i == p else 0 for i in range(n)])
                carrier = nc.sync.nop()
                wait_clock.add_sem_waits(carrier.ins, ScopedClock({None: vc}))
        nc.sync.drain()
        nc.all_engine_barrier()
        assert self.sems is not None
        popped = nc._tile_sem_poison_stack.pop()
        assert popped is self._sem_poison
        nc.clear_and_free_semaphores(list(self.sems.allocated().values()))
        nc.all_engine_barrier()

    MAXW = 1

    def _split_waits_in_blocks(self, ordered_blocks):
        nc = self.nc
        for bb_name, insts in ordered_blocks.items():
            new_list = []
            for inst in insts:
                si = inst.sync_info
                waits = list(si.on_wait) if (si and si.on_wait) else []
                if len(waits) > MAXW:
                    keep = waits[:MAXW - 1]
                    excess = waits[MAXW - 1:]
                    for k in range(0, len(excess), MAXW):
                        chunk = excess[k:k + MAXW]
                        carrier = mybir.InstEventSemaphore(
                            name=f"WSPLIT-{nc.next_id()}", ins=[], outs=[])
                        carrier.engine = inst.engine
                        carrier.sync_info = mybir.SyncInfo(
                            on_wait=list(chunk), on_update=[])
                        carrier.debug = inst.debug
                        new_list.append(carrier)
                    inst.sync_info = mybir.SyncInfo(
                        on_wait=keep,
                        on_update=list(si.on_update) if si.on_update else [])
                new_list.append(inst)
            insts[:] = new_list

    _orig_lower = tile.TileContext._lower_ordered_insts

    def _patched_lower(self, postordered_blocks):
        _split_waits_in_blocks(self, postordered_blocks)
        return _orig_lower(self, postordered_blocks)

    tile.TileContext._drain_and_barrier = _split_drain_and_barrier
    if getattr(tile.TileContext._lower_ordered_insts, "__name__", "") != "_patched_lower":
        tile.TileContext._lower_ordered_insts = _patched_lower

    _runtime["bass"] = bass
    _runtime["tile"] = tile
    _runtime["mybir"] = mybir
    return _runtime


def _finalize_libraries(nc, mybir):
    import bass_rust
    from concourse.library_config import all_libraries, standard
    mask = {}
    for lib in all_libraries:
        for t in lib.instructions:
            mask[t] = mask.get(t, 0) | (1 << lib.index)
    bass_rust.insert_library_loads(nc, mask, len(all_libraries), standard.index)
    mybir.codegen_inst_isa_subclasses(nc)


# --------------------------------------------------------------------------
# host-side preprocessing (index manipulation only)
# --------------------------------------------------------------------------

def _cascade_masks(lptr, ES):
    """Baseline boundary-extraction cascade masks (see kernel.py)."""
    W = PRE
    WA = PRE + ES + 1
    nparts, npd1 = lptr.shape
    g = np.empty((nparts, W), np.int64)
    g[:, :npd1] = PRE + lptr
    g[:, npd1:] = (PRE + lptr[:, -1:]) + np.arange(1, W - npd1 + 1)[None, :]
    d = np.arange(W)[None, :]
    o = g - d
    assert (o >= 0).all() and int(g.max()) < WA
    nbits = max(1, int(np.ceil(np.log2(int(o.max()) + 1))))
    pos = np.broadcast_to(d, (nparts, W)).copy()
    rowoff = (np.arange(nparts) * WA)[:, None]
    masks_by_shift = {}
    for j in range(nbits - 1, -1, -1):
        b = ((o >> j) & 1).astype(np.uint8)
        lo = np.full(nparts * WA, 2, np.int8)
        hi = np.full(nparts * WA, -1, np.int8)
        flat = (rowoff + pos).ravel()
        np.minimum.at(lo, flat, b.ravel().astype(np.int8))
        np.maximum.at(hi, flat, b.ravel().astype(np.int8))
        used = hi >= 0
        assert (lo[used] == hi[used]).all(), "cascade routing conflict"
        m = np.zeros(nparts * WA, np.uint8)
        m[used] = hi[used].astype(np.uint8)
        masks_by_shift[1 << j] = m.reshape(nparts, WA)
        pos = pos + (b.astype(np.int64) << j)
    assert (pos == g).all()
    shifts = sorted(masks_by_shift)
    masks = [masks_by_shift[s] for s in shifts]
    return shifts, masks


def _wrap_stream(pages, width):
    """[n] int stream -> [128, width*8] int16 wrapped in 16, replicated x8."""
    n = len(pages)
    k16 = width * 8
    pad = np.zeros(k16 * 16, np.int16)
    pad[:n] = pages.astype(np.int16)
    w = pad.reshape(k16, 16).T          # [16, k16]
    return np.tile(w, (8, 1))           # [128, k16]


def _host_prep(edge_index):
    row = np.asarray(edge_index[0], dtype=np.int64)
    col = np.asarray(edge_index[1], dtype=np.int64)
    deg = np.bincount(col, minlength=M).astype(np.float32) + 1.0

    percore = []
    for k in range(NCORES):
        sel = (col >= k * MC) & (col < (k + 1) * MC)
        r = row[sel]
        c = col[sel] - k * MC
        # zero-degree dests get a dummy zero-message slot: keeps lptr
        # strictly increasing, which the cascade mask builder requires.
        dcnt = np.bincount(c, minlength=MC)
        zdest = np.nonzero(dcnt == 0)[0]
        r = np.concatenate([r, np.full(len(zdest), -1, np.int64)])
        c = np.concatenate([c, zdest])
        order = np.argsort(c, kind="stable")
        r, c = r[order], c[order]
        part = c // PD
        cnt = np.bincount(part, minlength=P)
        starts = np.concatenate([[0], np.cumsum(cnt)])
        lptr = np.zeros((P, PD + 1), np.int64)
        slot = np.empty(len(c), np.int64)
        for p in range(P):
            sl = slice(starts[p], starts[p + 1])
            loc = c[sl] - p * PD
            lptr[p] = np.searchsorted(loc, np.arange(PD + 1))
            slot[sl] = np.arange(starts[p + 1] - starts[p])
        percore.append(dict(r=r, part=part, slot=slot, cnt=cnt, lptr=lptr))

    cnt_max = int(max(pc["cnt"].max() for pc in percore))
    ES = int(32 * np.ceil((cnt_max + 40) / 32))
    ES32 = ES // 32
    SMAX = cnt_max + 1                            # slots beyond are all-pad

    # uniform per-class round-1 lengths across cores
    dem_all = np.zeros((NCORES, 32, 32), np.int64)
    for k, pc in enumerate(percore):
        real = pc["r"] >= 0
        o = pc["r"][real] % 32
        kblk = pc["slot"][real] // ES32
        np.add.at(dem_all[k], (o, kblk), 1)
    Lo = np.ceil(dem_all.max(axis=(0, 2)) / 4).astype(np.int64)
    Lo = np.maximum(Lo, 1)
    b = np.concatenate([[0], np.cumsum(Lo)])
    S1 = int(b[-1])
    S1 += (1 - S1) % 32                    # S1 = 1 (mod 32)
    ZP = 4 * S1                            # first of 32 zero pages in staged
    assert ZP + 32 < 32768 and NPAGES < 32768

    cores = []
    all_shifts = None
    for k, pc in enumerate(percore):
        r, part, slot = pc["r"], pc["part"], pc["slot"]
        real = r >= 0
        o = r[real] % 32
        src_page = r[real] >> 5
        kblk = slot[real] // ES32
        p_dest = part[real]
        s_dest = slot[real]
        # rank within (o, kblk) group
        order2 = np.lexsort((np.arange(o.size), kblk, o))
        oo, kk = o[order2], kblk[order2]
        grp = oo * 32 + kk
        first = np.concatenate([[True], grp[1:] != grp[:-1]])
        gidx = np.cumsum(first) - 1
        gstart = np.nonzero(first)[0]
        j = np.arange(o.size) - gstart[gidx]
        assert (j < 4 * Lo[oo]).all(), "round-1 class capacity exceeded"
        s1 = b[oo] + (j >> 2)
        p1 = ((kk - s1) % 32) + 32 * (j & 3)
        # round-1 idx stream: pos i1 = s1*128 + p1 -> table page.
        # pad positions gather unused data; spread them over random pages
        # (a single shared pad page serializes on one DRAM bank).
        prng = np.random.default_rng(12345 + k)
        pages1 = prng.integers(0, NPAGES, S1 * 128).astype(np.int16)
        pages1[s1 * 128 + p1] = src_page[order2].astype(np.int16)
        # round-2: final (p, s) -> staged page; pads cycle over 32 zero pages
        flat1 = p1 * S1 + s1
        assert ((flat1 & 31) == kk).all()
        pages2 = (ZP + (np.arange(ES * 128) & 31)).astype(np.int16)
        i2 = s_dest[order2] * 128 + p_dest[order2]
        pages2[i2] = (flat1 >> 5).astype(np.int16)
        shifts, masks = _cascade_masks(pc["lptr"], ES)
        degf = None  # filled below
        cores.append(dict(pages1=pages1, pages2=pages2, shifts=shifts,
                          masks=masks))
    all_shifts = sorted({s for cd in cores for s in cd["shifts"]})
    WA = PRE + ES + 1
    for k, cd in enumerate(cores):
        sh2m = dict(zip(cd["shifts"], cd["masks"]))
        zero = np.zeros((P, WA), np.uint8)
        cd["masks"] = [sh2m.get(s, zero) for s in all_shifts]
        cd["shifts"] = all_shifts
        cd["degf"] = deg[k * MC:(k + 1) * MC].reshape(P, PD)
        cd["widx1"] = _wrap_stream(cd.pop("pages1"), S1)
        cd["widx2"] = _wrap_stream(cd.pop("pages2"), ES)
    return ES, S1, tuple(Lo.tolist()), all_shifts, SMAX, cores


# --------------------------------------------------------------------------
# device program
# --------------------------------------------------------------------------

_REG_CACHE = {}


def _num_idxs_reg(nc, n):
    cache = _REG_CACHE.setdefault(id(nc), {})
    if n not in cache:
        cache[n] = nc.gpsimd.to_reg(n)
    return cache[n]


def _emit_dma_gather(nc, mybir, out_ap, in_ap, idxs_ap, num_idxs, queue_num=0):
    """InstDMAGatherAnt with 8B elements (elem_size=2 f32, stride 256B)."""
    eng = nc.gpsimd
    _in_ap = eng.lower_ap_dma(in_ap, for_custom_bir_dma=True)
    _idxs_ap = eng.lower_ap(idxs_ap)
    _out_ap = eng.lower_ap(out_ap)
    return eng.add_instruction(
        mybir.InstDMAGatherAnt(
            name=nc.get_next_instruction_name(),
            ins=[*_in_ap, _idxs_ap,
                 eng.lower_val_access(_num_idxs_reg(nc, num_idxs))],
            outs=[_out_ap],
            transpose=False, num_idxs=num_idxs, elem_size=2,
            stride_bytes_256=1, gen_mode=0, single_packet=True,
            queue_num=queue_num, sbuf_tokens_per_rank=0,
            sbuf_free_dim_per_rank=0, sbuf_free_dim_pad_per_rank=0,
            sbuf_byte_offset=0,
        ))


def _build_program(ES, S1, Lo, shifts, SMAX):
    rt = _setup_runtime()
    bass, tile, mybir = rt["bass"], rt["tile"], rt["mybir"]
    f32, i16, u8 = mybir.dt.float32, mybir.dt.int16, mybir.dt.uint8
    bf16 = mybir.dt.bfloat16
    AF = mybir.ActivationFunctionType
    ALU = mybir.AluOpType
    nc = bass.Bass(target_bir_lowering=False, num_swdge_queues=NQUEUES)

    WA = PRE + ES + 1
    ES32 = ES // 32
    bcls = np.concatenate([[0], np.cumsum(np.asarray(Lo))]).astype(int)

    xloc = nc.declare_dram_parameter("xloc", [P, PD, 2], f32, isOutput=False)
    degf = nc.declare_dram_parameter("degf", [P, PD], f32, isOutput=False)
    widx1 = nc.declare_dram_parameter("widx1", [P, S1 * 8], i16, isOutput=False)
    widx2 = nc.declare_dram_parameter("widx2", [P, ES * 8], i16, isOutput=False)
    bmasks = nc.declare_dram_parameter("bmasks", [len(shifts), P, WA], u8,
                                       isOutput=False)
    w1 = nc.declare_dram_parameter("w1", [2, HID], f32, isOutput=False)
    gamma = nc.declare_dram_parameter("gamma", [1, HID], f32, isOutput=False)
    beta = nc.declare_dram_parameter("beta", [1, HID], f32, isOutput=False)
    w2 = nc.declare_dram_parameter("w2", [HID, 2], f32, isOutput=False)
    b2 = nc.declare_dram_parameter("b2", [1, 2], f32, isOutput=False)
    out_ext = nc.declare_dram_parameter("out", [P, PD, 2], f32, isOutput=True)

    shard = nc.dram_tensor("shard", [MC * 2], f32)
    table = nc.dram_tensor("table", [M * 2], f32, addr_space="Shared")
    staged = nc.dram_tensor("staged", [S1 * 128 * 2 + 64 * 32], f32)
    bn_in = nc.dram_tensor("bn_in", [2 * HID], f32)
    bn_out = nc.dram_tensor("bn_out", [2 * HID], f32, addr_space="Shared")
    groups = [list(range(NCORES))]

    from concourse.masks import make_identity

    with tile.TileContext(nc) as tc:
        with (
            tc.tile_pool(name="big", bufs=1) as big,
            tc.tile_pool(name="gst", bufs=3) as gst,
            tc.tile_pool(name="small", bufs=1) as small,
            tc.tile_pool(name="ps", bufs=2, space="PSUM") as psp,
        ):
            # xs -> publish -> AllGather is the critical chain: load its
            # inputs first so the collective starts ASAP; widx streams can
            # arrive while it runs.
            xl = big.tile([P, PD, 2], f32)
            nc.sync.dma_start(out=xl[:], in_=xloc[:])
            dg = big.tile([P, PD], f32)
            nc.sync.dma_start(out=dg[:], in_=degf[:])
            widx1_t = big.tile([P, S1 * 8], i16)
            nc.sync.dma_start(out=widx1_t[:], in_=widx1[:])
            widx2_t = big.tile([P, ES * 8], i16)
            nc.sync.dma_start(out=widx2_t[:], in_=widx2[:])

            def part_bcast(ap):
                return bass.AP(tensor=ap.tensor, offset=ap.offset,
                               ap=[[0, P], *ap.ap])

            w1_t = small.tile([P, 2 * HID], f32)
            nc.sync.dma_start(out=w1_t[:], in_=part_bcast(w1[:, :]))
            w2_t = small.tile([P, HID * 2], f32)
            nc.sync.dma_start(out=w2_t[:], in_=part_bcast(w2[:, :]))
            gm_t = small.tile([P, HID], f32)
            nc.sync.dma_start(out=gm_t[:], in_=part_bcast(gamma[0, :]))
            bt_t = small.tile([P, HID], f32)
            nc.sync.dma_start(out=bt_t[:], in_=part_bcast(beta[0, :]))
            b2_t = small.tile([P, 2], f32)
            nc.sync.dma_start(out=b2_t[:], in_=part_bcast(b2[0, :]))

            # 32 zero pages of the staged buffer
            zpg = small.tile([128, 16], f32)
            nc.vector.memset(zpg[:], 0.0)
            nc.sync.dma_start(out=staged[S1 * 256:S1 * 256 + 64 * 32],
                              in_=zpg[:])

            dinv = dg
            nc.scalar.activation(out=dinv[:], in_=dg[:], func=AF.Sqrt)
            nc.vector.reciprocal(out=dinv[:], in_=dinv[:])

            def bcast_pd2(t):
                a = t[:]
                return bass.AP(tensor=a.tensor, offset=a.offset,
                               ap=[a.ap[0], a.ap[1], [0, 2]])

            def mul_dinv(dst, src):
                nc.vector.tensor_tensor(out=dst[:], in0=src[:],
                                        in1=bcast_pd2(dinv), op=ALU.mult)

            stg1 = big.tile([P, S1, 2], f32)
            msg = big.tile([P, ES, 2], f32)
            A = big.tile([P, WA, 2], f32)
            agg = big.tile([P, PD, 2], f32)
            zero1 = small.tile([P, 2], f32)
            nc.vector.memset(zero1[:], 0.0)

            qctr = [0]

            def chunked_gather(dst, in_ap_fn, idxs_t, lo, hi):
                """gathers in <=CHUNK_SLOTS chunks, rotating queues."""
                s = lo
                while s < hi:
                    e = min(s + CHUNK_SLOTS, hi)
                    _emit_dma_gather(
                        nc, mybir, dst[:, s:e, :], in_ap_fn(),
                        idxs_t[:, s * 8:e * 8], (e - s) * 128,
                        queue_num=qctr[0] % NQUEUES)
                    qctr[0] += 1
                    s = e

            def gather_layer():
                # round 1: table -> class-blocked staging
                for o in range(32):
                    in_ap = lambda o=o: bass.AP(
                        tensor=table[:].tensor, offset=o * 2,
                        ap=[[64, NPAGES], [1, 2]])
                    chunked_gather(stg1, in_ap, widx1_t,
                                   int(bcls[o]), int(bcls[o + 1]))
                # barrier: all round-1 gather DMAs landed in stg1
                nc.gpsimd.drain()
                # staging -> DRAM, on gpsimd so the drain orders it
                st_ap = bass.AP(tensor=staged[:].tensor, offset=0,
                                ap=[[S1 * 2, P], [1, S1 * 2]])
                nc.gpsimd.dma_start(out=st_ap, in_=stg1[:])
                nc.gpsimd.drain()
                # round 2: staged -> dest-sorted msg. Slots >= SMAX are
                # all-pad in every partition; skip gathering them (the scan
                # may read garbage there but no boundary is extracted past
                # lptr <= cnt < SMAX).
                for kblk in range(32):
                    in_ap = lambda kblk=kblk: bass.AP(
                        tensor=staged[:].tensor, offset=kblk * 2,
                        ap=[[64, 4 * S1 + 32], [1, 2]])
                    hi = min((kblk + 1) * ES32, SMAX)
                    if kblk * ES32 >= hi:
                        continue
                    chunked_gather(msg, in_ap, widx2_t, kblk * ES32, hi)
                # barrier: all round-2 DMAs landed; then touch a pad slot of
                # msg on gpsimd so tile orders the vector scan after this
                # point (cross-engine visibility of the gathered data).
                nc.gpsimd.drain()
                nc.gpsimd.memset(msg[:, ES - 1:ES, :], 0.0)

            def aggregate(own):
                nc.vector.memset(A[:, :PRE + 1, :], 0.0)
                for f in range(2):
                    ma = msg[:]
                    src = bass.AP(tensor=ma.tensor, offset=ma.offset + f,
                                  ap=[ma.ap[0], [2, ES]])
                    aa = A[:]
                    dst = bass.AP(tensor=aa.tensor,
                                  offset=aa.offset + (PRE + 1) * 2 + f,
                                  ap=[aa.ap[0], [2, ES]])
                    zb = bass.AP(tensor=zero1.tensor, offset=zero1[:].offset,
                                 ap=[zero1[:].ap[0], [0, ES]])
                    nc.vector.tensor_tensor_scan(
                        out=dst, data0=src, data1=zb, initial=0.0,
                        op0=ALU.add, op1=ALU.add)
                for si, s in enumerate(shifts):
                    wdt = WA - s
                    mt = gst.tile([P, WA], u8, tag="cmask")
                    nc.sync.dma_start(out=mt[:], in_=bmasks[si])
                    mm = mt[:, :wdt]
                    mba = bass.AP(tensor=mm.tensor, offset=mm.offset,
                                  ap=[mm.ap[0], mm.ap[1], [0, 2]])
                    nc.vector.copy_predicated(
                        out=A[:, 0:wdt, :], mask=mba, data=A[:, s:s + wdt, :])
                nc.vector.tensor_tensor(out=agg[:], in0=A[:, 1:PD + 1, :],
                                        in1=A[:, 0:PD, :], op=ALU.subtract)
                nc.vector.tensor_tensor(out=agg[:], in0=agg[:], in1=own[:],
                                        op=ALU.add)
                mul_dinv(agg, agg)

            def publish(src):
                nc.sync.dma_start(out=shard[:], in_=src[:])
                return nc.gpsimd.collective_compute(
                    "AllGather", ALU.bypass, replica_groups=groups,
                    ins=[shard[:]], outs=[table[:]])

            # =========== layer 1 ===========
            xs = xl
            mul_dinv(xs, xl)
            publish(xs)
            gather_layer()
            aggregate(xs)

            h = big.tile([P, HID, PD], bf16)
            ag = agg[:]
            a0 = bass.AP(tensor=ag.tensor, offset=ag.offset, ap=[ag.ap[0], [2, PD]])
            a1 = bass.AP(tensor=ag.tensor, offset=ag.offset + 1, ap=[ag.ap[0], [2, PD]])
            for j in range(HID):
                nc.scalar.activation(out=h[:, j, :], in_=a0, func=AF.Copy,
                                     scale=w1_t[:, j:j + 1])
                nc.vector.scalar_tensor_tensor(
                    out=h[:, j, :], in0=a1, scalar=w1_t[:, HID + j:HID + j + 1],
                    in1=h[:, j, :], op0=ALU.mult, op1=ALU.add)

            st = small.tile([P, 2 * HID], f32)
            nc.vector.tensor_reduce(out=st[:, :HID], in_=h[:],
                                    axis=mybir.AxisListType.X, op=ALU.add)
            sqscratch = small.tile([P, PD], f32)
            for j in range(HID):
                nc.scalar.activation(
                    out=sqscratch[:], in_=h[:, j, :], func=AF.Square,
                    accum_out=st[:, HID + j:HID + j + 1])
            ones = small.tile([P, 1], f32)
            nc.vector.memset(ones[:], 1.0)
            stp = psp.tile([P, 2 * HID], f32, space="PSUM")
            nc.tensor.matmul(out=stp[:1, :], lhsT=ones[:], rhs=st[:],
                             start=True, stop=True)
            sred = small.tile([1, 2 * HID], f32)
            nc.vector.tensor_copy(out=sred[:], in_=stp[:1, :])
            nc.sync.dma_start(out=bn_in[:], in_=sred[:])
            nc.gpsimd.collective_compute(
                "AllReduce", ALU.add, replica_groups=groups,
                ins=[bn_in[:]], outs=[bn_out[:]])
            sums = small.tile([P, 2 * HID], f32)
            nc.sync.dma_start(out=sums[:], in_=part_bcast(bn_out[:]))
            mv = small.tile([P, 2 * HID], f32)
            nc.vector.tensor_scalar_mul(mv[:, :HID], sums[:, :HID], 1.0 / M)
            nc.vector.tensor_scalar_mul(mv[:, HID:], sums[:, HID:], 1.0 / M)
            nc.vector.tensor_tensor(out=sums[:, :HID], in0=mv[:, :HID],
                                    in1=mv[:, :HID], op=ALU.mult)
            nc.vector.tensor_tensor(out=mv[:, HID:], in0=mv[:, HID:],
                                    in1=sums[:, :HID], op=ALU.subtract)
            sbn = small.tile([P, 2 * HID], f32)
            nc.vector.tensor_scalar_add(mv[:, HID:], mv[:, HID:], BN_EPS)
            nc.scalar.activation(out=sbn[:, :HID], in_=mv[:, HID:], func=AF.Sqrt)
            nc.vector.reciprocal(out=sbn[:, :HID], in_=sbn[:, :HID])
            nc.vector.tensor_tensor(out=sbn[:, :HID], in0=sbn[:, :HID],
                                    in1=gm_t[:], op=ALU.mult)
            nc.vector.tensor_tensor(out=sbn[:, HID:], in0=mv[:, :HID],
                                    in1=sbn[:, :HID], op=ALU.mult)
            nc.vector.tensor_tensor(out=sbn[:, HID:], in0=bt_t[:],
                                    in1=sbn[:, HID:], op=ALU.subtract)

            for j in range(HID):
                nc.scalar.activation(out=h[:, j, :], in_=h[:, j, :],
                                     func=AF.Relu,
                                     scale=sbn[:, j:j + 1],
                                     bias=sbn[:, HID + j:HID + j + 1])
            ys = big.tile([P, PD, 2], f32)
            yv = ys[:]
            for f in range(2):
                yf = bass.AP(tensor=yv.tensor, offset=yv.offset + f,
                             ap=[yv.ap[0], [2, PD]])
                nc.scalar.activation(out=yf, in_=h[:, 0, :], func=AF.Copy,
                                     scale=w2_t[:, f:f + 1])
                for j in range(1, HID):
                    nc.vector.scalar_tensor_tensor(
                        out=yf, in0=h[:, j, :],
                        scalar=w2_t[:, j * 2 + f:j * 2 + f + 1],
                        in1=yf, op0=ALU.mult, op1=ALU.add)
            mul_dinv(ys, ys)

            # =========== layer 2 ===========
            publish(ys)
            gather_layer()
            aggregate(ys)
            b2b = bass.AP(tensor=b2_t.tensor, offset=b2_t[:].offset,
                          ap=[b2_t[:].ap[0], [0, PD], [1, 2]])
            nc.vector.tensor_tensor(out=agg[:], in0=agg[:], in1=b2b, op=ALU.add)
            nc.sync.dma_start(out=out_ext[:], in_=agg[:])

    _finalize_libraries(nc, mybir)
    return nc


_prog_cache = {}
LAST_EXEC_NS = None


def _install_ntff_shim():
    import sys as _sys
    import types, contextlib, ctypes
    if "antenv.axon_hooks" in _sys.modules:
        return
    try:
        import antenv.axon_hooks  # noqa: F401
        return
    except ImportError:
        pass
    so_path = "/opt/axon/libaxon_pjrt.so"

    def _make_hook():
        lib = ctypes.CDLL(so_path)
        if not hasattr(lib, "axon_start_nrt_profile"):
            return None
        lib.axon_start_nrt_profile.argtypes = [
            ctypes.POINTER(ctypes.c_int64), ctypes.c_size_t]
        lib.axon_start_nrt_profile.restype = ctypes.c_int64
        lib.axon_stop_nrt_profile.argtypes = [ctypes.c_char_p]
        lib.axon_stop_nrt_profile.restype = ctypes.c_int64

        @contextlib.contextmanager
        def _hook_cm(output_dir, device_ids):
            import jax
            jax.devices()
            if device_ids:
                ids = (ctypes.c_int64 * len(device_ids))(*device_ids)
                rc = lib.axon_start_nrt_profile(ids, len(device_ids))
            else:
                rc = lib.axon_start_nrt_profile(None, 0)
            if rc != 0:
                raise RuntimeError(f"axon_start_nrt_profile rc={rc}")
            try:
                yield
            finally:
                lib.axon_stop_nrt_profile(str(output_dir).encode())

        return _hook_cm

    hook = [None]

    def get_axon_ntff_profile_hook():
        if hook[0] is None:
            hook[0] = _make_hook()
        return hook[0]

    mod = types.ModuleType("antenv.axon_hooks")
    mod.get_axon_ntff_profile_hook = get_axon_ntff_profile_hook
    mod.set_axon_ntff_profile_hook = lambda h: hook.__setitem__(0, h)
    _sys.modules["antenv.axon_hooks"] = mod


def kernel(x, edge_index, W1, b1, gamma, beta, W2, b2):
    global LAST_EXEC_NS
    import os
    from concourse.bass_utils import run_bass_kernel_spmd

    x = np.asarray(x)
    xf = x.reshape(M, 2).astype(np.float32)
    ES, S1, Lo, shifts, SMAX, cores = _host_prep(np.asarray(edge_index))

    key = (ES, S1, Lo, tuple(shifts), SMAX)
    if key not in _prog_cache:
        _prog_cache[key] = _build_program(ES, S1, Lo, shifts, SMAX)
    nc = _prog_cache[key]

    in_maps = []
    for k in range(NCORES):
        cd = cores[k]
        in_maps.append({
            "xloc": xf[k * MC:(k + 1) * MC].reshape(P, PD, 2),
            "degf": cd["degf"],
            "widx1": cd["widx1"],
            "widx2": cd["widx2"],
            "bmasks": np.stack(cd["masks"]).astype(np.uint8),
            "w1": np.asarray(W1, np.float32),
            "gamma": np.asarray(gamma, np.float32).reshape(1, HID),
            "beta": np.asarray(beta, np.float32).reshape(1, HID),
            "w2": np.asarray(W2, np.float32),
            "b2": np.asarray(b2, np.float32).reshape(1, 2),
        })
    trace = os.environ.get("GCN_TRACE") == "1"
    if trace:
        _install_ntff_shim()
    res = None
    last_exc = None
    for attempt in range(3):
        try:
            res = run_bass_kernel_spmd(nc, in_maps, list(range(NCORES)),
                                       trace=trace)
            break
        except Exception as e:
            last_exc = e
            import time as _time
            _time.sleep(3.0)
    if res is None:
        raise last_exc
    if res.exec_time_ns is not None:
        LAST_EXEC_NS = res.exec_time_ns
    out = np.concatenate([res.results[k]["out"].reshape(MC, 2)
                          for k in range(NCORES)], axis=0)
    return out.reshape(N, T, L).astype(np.float32)



# revision 30
# speedup vs baseline: 1.7553x; 1.0044x over previous
"""GCN layer on 8 NeuronCores — two-round batched dma_gather version.

Per layer, per core (dest-sharded, MC=102400 dests):
  Round 1: 32 InstDMAGatherAnt instructions (one per src%32 offset class,
    hbm_base = table + o*8, stride 256B, elem 8B) fetch each edge's source
    row into a class-blocked staging buffer (stream pos i -> SBUF
    [i%128, i//128]).  Host assigns each edge a staging position whose
    DRAM flat index (p1*S1 + s1, S1 = 1 mod 32) has residue == the edge's
    dest-sorted slot block, so that
  Round 2: after one contiguous SBUF->DRAM write, 32 more gathers (class k
    reads staged + k*8) land every message at its dest-sorted slot
    (p = dest partition, s = dest-sorted rank).  Zero page serves dummy
    (zero-degree) and pad slots.
  Aggregation: prefix scan + mask-cascade boundary extraction (unchanged).
  BatchNorm stats AllReduce'd; xs/ys tables AllGather'd.
"""

import numpy as np

N, T, V = 64, 512, 25
L = 2 * V
M = N * T * V            # 819200 nodes
P = 128
NCORES = 8
MC = M // NCORES         # 102400 dests per core
PD = MC // P             # 800 dests per partition
NPAGES = M // 32         # 25600 table pages
HID = 20
BN_EPS = 1e-5
PRE = 832
NQUEUES = 4
CHUNK_SLOTS = 8            # slots (x128 idx) per gather instruction; HW caps num_idxs at 1024

_runtime = {}


def _setup_runtime():
    if _runtime:
        return _runtime
    import concourse.bass as bass
    import concourse.tile as tile
    from concourse import mybir
    import bass_rust
    from concourse.vector_clock import ScopedClock, VectorClock

    def _split_drain_and_barrier(self, tick_clock, wait_clock):
        nc = self.nc
        gc = tick_clock.global_clock
        n = len(gc)
        for p in range(n):
            t = gc[p]
            if t > 0:
                vc = VectorClock([t if i == p else 0 for i in range(n)])
                carrier = nc.sync.nop()
                wait_clock.add_sem_waits(carrier.ins, ScopedClock({None: vc}))
        nc.sync.drain()
        nc.all_engine_barrier()
        assert self.sems is not None
        popped = nc._tile_sem_poison_stack.pop()
        assert popped is self._sem_poison
        nc.clear_and_free_semaphores(list(self.sems.allocated().values()))
        nc.all_engine_barrier()

    MAXW = 1

    def _split_waits_in_blocks(self, ordered_blocks):
        nc = self.nc
        for bb_name, insts in ordered_blocks.items():
            new_list = []
            for inst in insts:
                si = inst.sync_info
                waits = list(si.on_wait) if (si and si.on_wait) else []
                if len(waits) > MAXW:
                    keep = waits[:MAXW - 1]
                    excess = waits[MAXW - 1:]
                    for k in range(0, len(excess), MAXW):
                        chunk = excess[k:k + MAXW]
                        carrier = mybir.InstEventSemaphore(
                            name=f"WSPLIT-{nc.next_id()}", ins=[], outs=[])
                        carrier.engine = inst.engine
                        carrier.sync_info = mybir.SyncInfo(
                            on_wait=list(chunk), on_update=[])
                        carrier.debug = inst.debug
                        new_list.append(carrier)
                    inst.sync_info = mybir.SyncInfo(
                        on_wait=keep,
                        on_update=list(si.on_update) if si.on_update else [])
                new_list.append(inst)
            insts[:] = new_list

    _orig_lower = tile.TileContext._lower_ordered_insts

    def _patched_lower(self, postordered_blocks):
        _split_waits_in_blocks(self, postordered_blocks)
        return _orig_lower(self, postordered_blocks)

    tile.TileContext._drain_and_barrier = _split_drain_and_barrier
    if getattr(tile.TileContext._lower_ordered_insts, "__name__", "") != "_patched_lower":
        tile.TileContext._lower_ordered_insts = _patched_lower

    _runtime["bass"] = bass
    _runtime["tile"] = tile
    _runtime["mybir"] = mybir
    return _runtime


def _finalize_libraries(nc, mybir):
    import bass_rust
    from concourse.library_config import all_libraries, standard
    mask = {}
    for lib in all_libraries:
        for t in lib.instructions:
            mask[t] = mask.get(t, 0) | (1 << lib.index)
    bass_rust.insert_library_loads(nc, mask, len(all_libraries), standard.index)
    mybir.codegen_inst_isa_subclasses(nc)


# --------------------------------------------------------------------------
# host-side preprocessing (index manipulation only)
# --------------------------------------------------------------------------

def _cascade_masks(lptr, ES):
    """Baseline boundary-extraction cascade masks (see kernel.py)."""
    W = PRE
    WA = PRE + ES + 1
    nparts, npd1 = lptr.shape
    g = np.empty((nparts, W), np.int64)
    g[:, :npd1] = PRE + lptr
    g[:, npd1:] = (PRE + lptr[:, -1:]) + np.arange(1, W - npd1 + 1)[None, :]
    d = np.arange(W)[None, :]
    o = g - d
    assert (o >= 0).all() and int(g.max()) < WA
    nbits = max(1, int(np.ceil(np.log2(int(o.max()) + 1))))
    pos = np.broadcast_to(d, (nparts, W)).copy()
    rowoff = (np.arange(nparts) * WA)[:, None]
    masks_by_shift = {}
    for j in range(nbits - 1, -1, -1):
        b = ((o >> j) & 1).astype(np.uint8)
        lo = np.full(nparts * WA, 2, np.int8)
        hi = np.full(nparts * WA, -1, np.int8)
        flat = (rowoff + pos).ravel()
        np.minimum.at(lo, flat, b.ravel().astype(np.int8))
        np.maximum.at(hi, flat, b.ravel().astype(np.int8))
        used = hi >= 0
        assert (lo[used] == hi[used]).all(), "cascade routing conflict"
        m = np.zeros(nparts * WA, np.uint8)
        m[used] = hi[used].astype(np.uint8)
        masks_by_shift[1 << j] = m.reshape(nparts, WA)
        pos = pos + (b.astype(np.int64) << j)
    assert (pos == g).all()
    shifts = sorted(masks_by_shift)
    masks = [masks_by_shift[s] for s in shifts]
    return shifts, masks


def _wrap_stream(pages, width):
    """[n] int stream -> [128, width*8] int16 wrapped in 16, replicated x8."""
    n = len(pages)
    k16 = width * 8
    pad = np.zeros(k16 * 16, np.int16)
    pad[:n] = pages.astype(np.int16)
    w = pad.reshape(k16, 16).T          # [16, k16]
    return np.tile(w, (8, 1))           # [128, k16]


def _host_prep(edge_index):
    row = np.asarray(edge_index[0], dtype=np.int64)
    col = np.asarray(edge_index[1], dtype=np.int64)
    deg = np.bincount(col, minlength=M).astype(np.float32) + 1.0

    percore = []
    for k in range(NCORES):
        sel = (col >= k * MC) & (col < (k + 1) * MC)
        r = row[sel]
        c = col[sel] - k * MC
        # zero-degree dests get a dummy zero-message slot: keeps lptr
        # strictly increasing, which the cascade mask builder requires.
        dcnt = np.bincount(c, minlength=MC)
        zdest = np.nonzero(dcnt == 0)[0]
        r = np.concatenate([r, np.full(len(zdest), -1, np.int64)])
        c = np.concatenate([c, zdest])
        order = np.argsort(c, kind="stable")
        r, c = r[order], c[order]
        part = c // PD
        cnt = np.bincount(part, minlength=P)
        starts = np.concatenate([[0], np.cumsum(cnt)])
        lptr = np.zeros((P, PD + 1), np.int64)
        slot = np.empty(len(c), np.int64)
        for p in range(P):
            sl = slice(starts[p], starts[p + 1])
            loc = c[sl] - p * PD
            lptr[p] = np.searchsorted(loc, np.arange(PD + 1))
            slot[sl] = np.arange(starts[p + 1] - starts[p])
        percore.append(dict(r=r, part=part, slot=slot, cnt=cnt, lptr=lptr))

    cnt_max = int(max(pc["cnt"].max() for pc in percore))
    ES = int(32 * np.ceil((cnt_max + 40) / 32))
    ES32 = ES // 32
    SMAX = cnt_max + 1                            # slots beyond are all-pad

    # uniform per-class round-1 lengths across cores
    dem_all = np.zeros((NCORES, 32, 32), np.int64)
    for k, pc in enumerate(percore):
        real = pc["r"] >= 0
        o = pc["r"][real] % 32
        kblk = pc["slot"][real] // ES32
        np.add.at(dem_all[k], (o, kblk), 1)
    Lo = np.ceil(dem_all.max(axis=(0, 2)) / 4).astype(np.int64)
    Lo = np.maximum(Lo, 1)
    b = np.concatenate([[0], np.cumsum(Lo)])
    S1 = int(b[-1])
    S1 += (1 - S1) % 32                    # S1 = 1 (mod 32)
    ZP = 4 * S1                            # first of 32 zero pages in staged
    assert ZP + 32 < 32768 and NPAGES < 32768

    cores = []
    all_shifts = None
    for k, pc in enumerate(percore):
        r, part, slot = pc["r"], pc["part"], pc["slot"]
        real = r >= 0
        o = r[real] % 32
        src_page = r[real] >> 5
        kblk = slot[real] // ES32
        p_dest = part[real]
        s_dest = slot[real]
        # rank within (o, kblk) group
        order2 = np.lexsort((np.arange(o.size), kblk, o))
        oo, kk = o[order2], kblk[order2]
        grp = oo * 32 + kk
        first = np.concatenate([[True], grp[1:] != grp[:-1]])
        gidx = np.cumsum(first) - 1
        gstart = np.nonzero(first)[0]
        j = np.arange(o.size) - gstart[gidx]
        assert (j < 4 * Lo[oo]).all(), "round-1 class capacity exceeded"
        s1 = b[oo] + (j >> 2)
        p1 = ((kk - s1) % 32) + 32 * (j & 3)
        # round-1 idx stream: pos i1 = s1*128 + p1 -> table page.
        # pad positions gather unused data; spread them over random pages
        # (a single shared pad page serializes on one DRAM bank).
        prng = np.random.default_rng(12345 + k)
        pages1 = prng.integers(0, NPAGES, S1 * 128).astype(np.int16)
        pages1[s1 * 128 + p1] = src_page[order2].astype(np.int16)
        # round-2: final (p, s) -> staged page; pads cycle over 32 zero pages
        flat1 = p1 * S1 + s1
        assert ((flat1 & 31) == kk).all()
        pages2 = (ZP + (np.arange(ES * 128) & 31)).astype(np.int16)
        i2 = s_dest[order2] * 128 + p_dest[order2]
        pages2[i2] = (flat1 >> 5).astype(np.int16)
        shifts, masks = _cascade_masks(pc["lptr"], ES)
        degf = None  # filled below
        cores.append(dict(pages1=pages1, pages2=pages2, shifts=shifts,
                          masks=masks))
    all_shifts = sorted({s for cd in cores for s in cd["shifts"]})
    WA = PRE + ES + 1
    for k, cd in enumerate(cores):
        sh2m = dict(zip(cd["shifts"], cd["masks"]))
        zero = np.zeros((P, WA), np.uint8)
        cd["masks"] = [sh2m.get(s, zero) for s in all_shifts]
        cd["shifts"] = all_shifts
        cd["degf"] = deg[k * MC:(k + 1) * MC].reshape(P, PD)
        cd["widx1"] = _wrap_stream(cd.pop("pages1"), S1)
        cd["widx2"] = _wrap_stream(cd.pop("pages2"), ES)
    return ES, S1, tuple(Lo.tolist()), all_shifts, SMAX, cores


# --------------------------------------------------------------------------
# device program
# --------------------------------------------------------------------------

_REG_CACHE = {}


def _num_idxs_reg(nc, n):
    cache = _REG_CACHE.setdefault(id(nc), {})
    if n not in cache:
        cache[n] = nc.gpsimd.to_reg(n)
    return cache[n]


def _emit_dma_gather(nc, mybir, out_ap, in_ap, idxs_ap, num_idxs, queue_num=0):
    """InstDMAGatherAnt with 8B elements (elem_size=2 f32, stride 256B)."""
    eng = nc.gpsimd
    _in_ap = eng.lower_ap_dma(in_ap, for_custom_bir_dma=True)
    _idxs_ap = eng.lower_ap(idxs_ap)
    _out_ap = eng.lower_ap(out_ap)
    return eng.add_instruction(
        mybir.InstDMAGatherAnt(
            name=nc.get_next_instruction_name(),
            ins=[*_in_ap, _idxs_ap,
                 eng.lower_val_access(_num_idxs_reg(nc, num_idxs))],
            outs=[_out_ap],
            transpose=False, num_idxs=num_idxs, elem_size=2,
            stride_bytes_256=1, gen_mode=0, single_packet=True,
            queue_num=queue_num, sbuf_tokens_per_rank=0,
            sbuf_free_dim_per_rank=0, sbuf_free_dim_pad_per_rank=0,
            sbuf_byte_offset=0,
        ))


def _build_program(ES, S1, Lo, shifts, SMAX):
    rt = _setup_runtime()
    bass, tile, mybir = rt["bass"], rt["tile"], rt["mybir"]
    f32, i16, u8 = mybir.dt.float32, mybir.dt.int16, mybir.dt.uint8
    bf16 = mybir.dt.bfloat16
    AF = mybir.ActivationFunctionType
    ALU = mybir.AluOpType
    nc = bass.Bass(target_bir_lowering=False, num_swdge_queues=NQUEUES)

    WA = PRE + ES + 1
    ES32 = ES // 32
    bcls = np.concatenate([[0], np.cumsum(np.asarray(Lo))]).astype(int)

    xloc = nc.declare_dram_parameter("xloc", [P, PD, 2], f32, isOutput=False)
    degf = nc.declare_dram_parameter("degf", [P, PD], f32, isOutput=False)
    widx1 = nc.declare_dram_parameter("widx1", [P, S1 * 8], i16, isOutput=False)
    widx2 = nc.declare_dram_parameter("widx2", [P, ES * 8], i16, isOutput=False)
    bmasks = nc.declare_dram_parameter("bmasks", [len(shifts), P, WA], u8,
                                       isOutput=False)
    w1 = nc.declare_dram_parameter("w1", [2, HID], f32, isOutput=False)
    gamma = nc.declare_dram_parameter("gamma", [1, HID], f32, isOutput=False)
    beta = nc.declare_dram_parameter("beta", [1, HID], f32, isOutput=False)
    w2 = nc.declare_dram_parameter("w2", [HID, 2], f32, isOutput=False)
    b2 = nc.declare_dram_parameter("b2", [1, 2], f32, isOutput=False)
    out_ext = nc.declare_dram_parameter("out", [P, PD, 2], f32, isOutput=True)

    shard = nc.dram_tensor("shard", [MC * 2], f32)
    table = nc.dram_tensor("table", [M * 2], f32, addr_space="Shared")
    staged = nc.dram_tensor("staged", [S1 * 128 * 2 + 64 * 32], f32)
    bn_in = nc.dram_tensor("bn_in", [2 * HID], f32)
    bn_out = nc.dram_tensor("bn_out", [2 * HID], f32, addr_space="Shared")
    groups = [list(range(NCORES))]

    from concourse.masks import make_identity

    with tile.TileContext(nc) as tc:
        with (
            tc.tile_pool(name="big", bufs=1) as big,
            tc.tile_pool(name="gst", bufs=3) as gst,
            tc.tile_pool(name="small", bufs=1) as small,
            tc.tile_pool(name="ps", bufs=2, space="PSUM") as psp,
        ):
            # xs -> publish -> AllGather is the critical chain: load its
            # inputs first so the collective starts ASAP; widx streams can
            # arrive while it runs.
            xl = big.tile([P, PD, 2], f32)
            nc.sync.dma_start(out=xl[:], in_=xloc[:])
            dg = big.tile([P, PD], f32)
            nc.sync.dma_start(out=dg[:], in_=degf[:])
            widx1_t = big.tile([P, S1 * 8], i16)
            nc.sync.dma_start(out=widx1_t[:], in_=widx1[:])
            widx2_t = big.tile([P, ES * 8], i16)
            nc.sync.dma_start(out=widx2_t[:], in_=widx2[:])

            def part_bcast(ap):
                return bass.AP(tensor=ap.tensor, offset=ap.offset,
                               ap=[[0, P], *ap.ap])

            w1_t = small.tile([P, 2 * HID], f32)
            nc.sync.dma_start(out=w1_t[:], in_=part_bcast(w1[:, :]))
            w2_t = small.tile([P, HID * 2], f32)
            nc.sync.dma_start(out=w2_t[:], in_=part_bcast(w2[:, :]))
            gm_t = small.tile([P, HID], f32)
            nc.sync.dma_start(out=gm_t[:], in_=part_bcast(gamma[0, :]))
            bt_t = small.tile([P, HID], f32)
            nc.sync.dma_start(out=bt_t[:], in_=part_bcast(beta[0, :]))
            b2_t = small.tile([P, 2], f32)
            nc.sync.dma_start(out=b2_t[:], in_=part_bcast(b2[0, :]))

            # 32 zero pages of the staged buffer
            zpg = small.tile([128, 16], f32)
            nc.vector.memset(zpg[:], 0.0)
            nc.sync.dma_start(out=staged[S1 * 256:S1 * 256 + 64 * 32],
                              in_=zpg[:])

            dinv = dg
            nc.scalar.activation(out=dinv[:], in_=dg[:], func=AF.Sqrt)
            nc.vector.reciprocal(out=dinv[:], in_=dinv[:])

            def bcast_pd2(t):
                a = t[:]
                return bass.AP(tensor=a.tensor, offset=a.offset,
                               ap=[a.ap[0], a.ap[1], [0, 2]])

            def mul_dinv(dst, src):
                nc.vector.tensor_tensor(out=dst[:], in0=src[:],
                                        in1=bcast_pd2(dinv), op=ALU.mult)

            stg1 = big.tile([P, S1, 2], f32)
            msg = big.tile([P, ES, 2], f32)
            A = big.tile([P, WA, 2], f32)
            agg = big.tile([P, PD, 2], f32)
            zero1 = small.tile([P, 2], f32)
            nc.vector.memset(zero1[:], 0.0)

            qctr = [0]

            def chunked_gather(dst, in_ap_fn, idxs_t, lo, hi):
                """gathers in <=CHUNK_SLOTS chunks, rotating queues."""
                s = lo
                while s < hi:
                    e = min(s + CHUNK_SLOTS, hi)
                    _emit_dma_gather(
                        nc, mybir, dst[:, s:e, :], in_ap_fn(),
                        idxs_t[:, s * 8:e * 8], (e - s) * 128,
                        queue_num=qctr[0] % NQUEUES)
                    qctr[0] += 1
                    s = e

            def gather_layer():
                # round 1: table -> class-blocked staging
                for o in range(32):
                    in_ap = lambda o=o: bass.AP(
                        tensor=table[:].tensor, offset=o * 2,
                        ap=[[64, NPAGES], [1, 2]])
                    chunked_gather(stg1, in_ap, widx1_t,
                                   int(bcls[o]), int(bcls[o + 1]))
                # barrier: all round-1 gather DMAs landed in stg1
                nc.gpsimd.drain()
                # staging -> DRAM, on gpsimd so the drain orders it
                st_ap = bass.AP(tensor=staged[:].tensor, offset=0,
                                ap=[[S1 * 2, P], [1, S1 * 2]])
                nc.gpsimd.dma_start(out=st_ap, in_=stg1[:])
                nc.gpsimd.drain()
                # round 2: staged -> dest-sorted msg. Slots >= SMAX are
                # all-pad in every partition; skip gathering them (the scan
                # may read garbage there but no boundary is extracted past
                # lptr <= cnt < SMAX).
                for kblk in range(32):
                    in_ap = lambda kblk=kblk: bass.AP(
                        tensor=staged[:].tensor, offset=kblk * 2,
                        ap=[[64, 4 * S1 + 32], [1, 2]])
                    hi = min((kblk + 1) * ES32, SMAX)
                    if kblk * ES32 >= hi:
                        continue
                    chunked_gather(msg, in_ap, widx2_t, kblk * ES32, hi)
                # barrier: all round-2 DMAs landed; then touch a pad slot of
                # msg on gpsimd so tile orders the vector scan after this
                # point (cross-engine visibility of the gathered data).
                nc.gpsimd.drain()
                nc.gpsimd.memset(msg[:, ES - 1:ES, :], 0.0)

            def aggregate(own):
                nc.vector.memset(A[:, :PRE + 1, :], 0.0)
                for f in range(2):
                    ma = msg[:]
                    src = bass.AP(tensor=ma.tensor, offset=ma.offset + f,
                                  ap=[ma.ap[0], [2, ES]])
                    aa = A[:]
                    dst = bass.AP(tensor=aa.tensor,
                                  offset=aa.offset + (PRE + 1) * 2 + f,
                                  ap=[aa.ap[0], [2, ES]])
                    zb = bass.AP(tensor=zero1.tensor, offset=zero1[:].offset,
                                 ap=[zero1[:].ap[0], [0, ES]])
                    nc.vector.tensor_tensor_scan(
                        out=dst, data0=src, data1=zb, initial=0.0,
                        op0=ALU.add, op1=ALU.add)
                for si, s in enumerate(shifts):
                    wdt = WA - s
                    mt = gst.tile([P, WA], u8, tag="cmask")
                    nc.sync.dma_start(out=mt[:], in_=bmasks[si])
                    mm = mt[:, :wdt]
                    mba = bass.AP(tensor=mm.tensor, offset=mm.offset,
                                  ap=[mm.ap[0], mm.ap[1], [0, 2]])
                    nc.vector.copy_predicated(
                        out=A[:, 0:wdt, :], mask=mba, data=A[:, s:s + wdt, :])
                nc.vector.tensor_tensor(out=agg[:], in0=A[:, 1:PD + 1, :],
                                        in1=A[:, 0:PD, :], op=ALU.subtract)
                nc.vector.tensor_tensor(out=agg[:], in0=agg[:], in1=own[:],
                                        op=ALU.add)
                mul_dinv(agg, agg)

            def publish(src):
                nc.sync.dma_start(out=shard[:], in_=src[:])
                return nc.gpsimd.collective_compute(
                    "AllGather", ALU.bypass, replica_groups=groups,
                    ins=[shard[:]], outs=[table[:]])

            # =========== layer 1 ===========
            xs = xl
            mul_dinv(xs, xl)
            publish(xs)
            gather_layer()
            aggregate(xs)

            h = big.tile([P, HID, PD], bf16)
            ag = agg[:]
            a0 = bass.AP(tensor=ag.tensor, offset=ag.offset, ap=[ag.ap[0], [2, PD]])
            a1 = bass.AP(tensor=ag.tensor, offset=ag.offset + 1, ap=[ag.ap[0], [2, PD]])
            for j in range(HID):
                nc.scalar.activation(out=h[:, j, :], in_=a0, func=AF.Copy,
                                     scale=w1_t[:, j:j + 1])
                nc.vector.scalar_tensor_tensor(
                    out=h[:, j, :], in0=a1, scalar=w1_t[:, HID + j:HID + j + 1],
                    in1=h[:, j, :], op0=ALU.mult, op1=ALU.add)

            st = small.tile([P, 2 * HID], f32)
            nc.vector.tensor_reduce(out=st[:, :HID], in_=h[:],
                                    axis=mybir.AxisListType.X, op=ALU.add)
            sqscratch = small.tile([P, PD], f32)
            for j in range(HID):
                nc.scalar.activation(
                    out=sqscratch[:], in_=h[:, j, :], func=AF.Square,
                    accum_out=st[:, HID + j:HID + j + 1])
            ones = small.tile([P, 1], f32)
            nc.vector.memset(ones[:], 1.0)
            stp = psp.tile([P, 2 * HID], f32, space="PSUM")
            nc.tensor.matmul(out=stp[:1, :], lhsT=ones[:], rhs=st[:],
                             start=True, stop=True)
            sred = small.tile([1, 2 * HID], f32)
            nc.vector.tensor_copy(out=sred[:], in_=stp[:1, :])
            nc.sync.dma_start(out=bn_in[:], in_=sred[:])
            nc.gpsimd.collective_compute(
                "AllReduce", ALU.add, replica_groups=groups,
                ins=[bn_in[:]], outs=[bn_out[:]])
            sums = small.tile([P, 2 * HID], f32)
            nc.sync.dma_start(out=sums[:], in_=part_bcast(bn_out[:]))
            mv = small.tile([P, 2 * HID], f32)
            nc.vector.tensor_scalar_mul(mv[:, :HID], sums[:, :HID], 1.0 / M)
            nc.vector.tensor_scalar_mul(mv[:, HID:], sums[:, HID:], 1.0 / M)
            nc.vector.tensor_tensor(out=sums[:, :HID], in0=mv[:, :HID],
                                    in1=mv[:, :HID], op=ALU.mult)
            nc.vector.tensor_tensor(out=mv[:, HID:], in0=mv[:, HID:],
                                    in1=sums[:, :HID], op=ALU.subtract)
            sbn = small.tile([P, 2 * HID], f32)
            nc.vector.tensor_scalar_add(mv[:, HID:], mv[:, HID:], BN_EPS)
            nc.scalar.activation(out=sbn[:, :HID], in_=mv[:, HID:], func=AF.Sqrt)
            nc.vector.reciprocal(out=sbn[:, :HID], in_=sbn[:, :HID])
            nc.vector.tensor_tensor(out=sbn[:, :HID], in0=sbn[:, :HID],
                                    in1=gm_t[:], op=ALU.mult)
            nc.vector.tensor_tensor(out=sbn[:, HID:], in0=mv[:, :HID],
                                    in1=sbn[:, :HID], op=ALU.mult)
            nc.vector.tensor_tensor(out=sbn[:, HID:], in0=bt_t[:],
                                    in1=sbn[:, HID:], op=ALU.subtract)

            for j in range(HID):
                nc.scalar.activation(out=h[:, j, :], in_=h[:, j, :],
                                     func=AF.Relu,
                                     scale=sbn[:, j:j + 1],
                                     bias=sbn[:, HID + j:HID + j + 1])
            ys = big.tile([P, PD, 2], f32)
            yv = ys[:]
            for f in range(2):
                yf = bass.AP(tensor=yv.tensor, offset=yv.offset + f,
                             ap=[yv.ap[0], [2, PD]])
                nc.scalar.activation(out=yf, in_=h[:, 0, :], func=AF.Copy,
                                     scale=w2_t[:, f:f + 1])
                for j in range(1, HID):
                    nc.vector.scalar_tensor_tensor(
                        out=yf, in0=h[:, j, :],
                        scalar=w2_t[:, j * 2 + f:j * 2 + f + 1],
                        in1=yf, op0=ALU.mult, op1=ALU.add)
            mul_dinv(ys, ys)

            # =========== layer 2 ===========
            publish(ys)
            gather_layer()
            aggregate(ys)
            b2b = bass.AP(tensor=b2_t.tensor, offset=b2_t[:].offset,
                          ap=[b2_t[:].ap[0], [0, PD], [1, 2]])
            nc.vector.tensor_tensor(out=agg[:], in0=agg[:], in1=b2b, op=ALU.add)
            nc.sync.dma_start(out=out_ext[:], in_=agg[:])

    _finalize_libraries(nc, mybir)
    return nc


_prog_cache = {}
LAST_EXEC_NS = None


def _install_ntff_shim():
    import sys as _sys
    import types, contextlib, ctypes
    if "antenv.axon_hooks" in _sys.modules:
        return
    try:
        import antenv.axon_hooks  # noqa: F401
        return
    except ImportError:
        pass
    so_path = "/opt/axon/libaxon_pjrt.so"

    def _make_hook():
        lib = ctypes.CDLL(so_path)
        if not hasattr(lib, "axon_start_nrt_profile"):
            return None
        lib.axon_start_nrt_profile.argtypes = [
            ctypes.POINTER(ctypes.c_int64), ctypes.c_size_t]
        lib.axon_start_nrt_profile.restype = ctypes.c_int64
        lib.axon_stop_nrt_profile.argtypes = [ctypes.c_char_p]
        lib.axon_stop_nrt_profile.restype = ctypes.c_int64

        @contextlib.contextmanager
        def _hook_cm(output_dir, device_ids):
            import jax
            jax.devices()
            if device_ids:
                ids = (ctypes.c_int64 * len(device_ids))(*device_ids)
                rc = lib.axon_start_nrt_profile(ids, len(device_ids))
            else:
                rc = lib.axon_start_nrt_profile(None, 0)
            if rc != 0:
                raise RuntimeError(f"axon_start_nrt_profile rc={rc}")
            try:
                yield
            finally:
                lib.axon_stop_nrt_profile(str(output_dir).encode())

        return _hook_cm

    hook = [None]

    def get_axon_ntff_profile_hook():
        if hook[0] is None:
            hook[0] = _make_hook()
        return hook[0]

    mod = types.ModuleType("antenv.axon_hooks")
    mod.get_axon_ntff_profile_hook = get_axon_ntff_profile_hook
    mod.set_axon_ntff_profile_hook = lambda h: hook.__setitem__(0, h)
    _sys.modules["antenv.axon_hooks"] = mod


def kernel(x, edge_index, W1, b1, gamma, beta, W2, b2):
    global LAST_EXEC_NS
    import os
    from concourse.bass_utils import run_bass_kernel_spmd

    x = np.asarray(x)
    xf = x.reshape(M, 2).astype(np.float32)
    ES, S1, Lo, shifts, SMAX, cores = _host_prep(np.asarray(edge_index))

    key = (ES, S1, Lo, tuple(shifts), SMAX)
    if key not in _prog_cache:
        _prog_cache[key] = _build_program(ES, S1, Lo, shifts, SMAX)
    nc = _prog_cache[key]

    in_maps = []
    for k in range(NCORES):
        cd = cores[k]
        in_maps.append({
            "xloc": xf[k * MC:(k + 1) * MC].reshape(P, PD, 2),
            "degf": cd["degf"],
            "widx1": cd["widx1"],
            "widx2": cd["widx2"],
            "bmasks": np.stack(cd["masks"]).astype(np.uint8),
            "w1": np.asarray(W1, np.float32),
            "gamma": np.asarray(gamma, np.float32).reshape(1, HID),
            "beta": np.asarray(beta, np.float32).reshape(1, HID),
            "w2": np.asarray(W2, np.float32),
            "b2": np.asarray(b2, np.float32).reshape(1, 2),
        })
    trace = os.environ.get("GCN_TRACE") == "1"
    if trace:
        _install_ntff_shim()
    res = None
    last_exc = None
    for attempt in range(3):
        try:
            res = run_bass_kernel_spmd(nc, in_maps, list(range(NCORES)),
                                       trace=trace)
            break
        except Exception as e:
            last_exc = e
            import time as _time
            _time.sleep(3.0)
    if res is None:
        raise last_exc
    if res.exec_time_ns is not None:
        LAST_EXEC_NS = res.exec_time_ns
    out = np.concatenate([res.results[k]["out"].reshape(MC, 2)
                          for k in range(NCORES)], axis=0)
    return out.reshape(N, T, L).astype(np.float32)

